# revision 74
# baseline (speedup 1.0000x reference)
"""Trainium2 Bass kernel for EnhancedGraphSAGE (embed -> 2x SAGE-mean -> GAT -> MLP).

Self-contained: takes full inputs, shards node-wise across 8 NeuronCores
internally, returns the full [N, C] output.

Design:
- Nodes are relabeled by a random permutation into NID = 8*56*128 internal ids
  (core-major, then 128-dst "blocks"). Each core owns its 56 blocks' dsts.
- Edges grouped by dst block; slots are padded to 128-wide tiles and expanded
  by dma_gather of 256B table rows, then aggregated per dst with TensorE
  matmuls against fp8 one-hot masks (lhsT = mask [128 slots, 128 dsts]).
- SAGE tables pack 2 nodes per 256B row (parity of src selects the 64-col
  window in the matmul), so idx = src//2 always fits int16 and the compact
  AllGather payload needs only a cheap local repack. SAGE masks are generated
  on-chip (DVE is_equal vs an iota) from 2-byte dst codes instead of loading
  14.7MB of one-hot masks per layer.
- AllGather payloads are fp8 (features; GAT also carries 4 bf16 el values in
  the 72B row), and each AG is split in two chunks: chunk 0 (each core's
  first half of rows) fires mid-aggregation of the previous layer so its
  transfer overlaps remaining gather work. GAT additionally keeps a unified
  "early" table of chunk-0 rows so early gather tiles can proceed before
  chunk 1 lands (tiles: 6 early + 5+5 late per block, grouped by src half
  for int16 range).
- GAT: softmax without max-subtraction (exp of leaky_relu bounded; leaky via
  ACT Prelu which shares the exp table set); er[dst] broadcast to edges via
  maskT matmul; z gets a 1e-20 floor via an extra PE accumulate row; per-head
  ex weighting on DVE; Wg folded into W1 on the host (U = Wg_h @ W1_h).
- Embed runs replicated (x in bf16, SWDGE-batched loads), writing the packed
  sage1 table directly.
"""

import numpy as np

import concourse.bacc as bacc
import concourse.bass as bass
import concourse.mybir as mybir
import concourse.tile as tile
from concourse.bass_utils import run_bass_kernel_spmd
from concourse.masks import make_identity

# Problem constants (hardcoded per spec)
N, E, IN, H, HEADS, C = 50000, 800000, 128, 64, 4, 40
SLOPE = 0.2

# Sharding geometry
NCORES = 8
NBLK = 56              # dst blocks per core
PB = 128               # dst slots per block
TPH = 8                # gather tiles per half (1024 idx limit of dma_gather)
TPB = 2 * TPH          # tiles per block
TE = 3                 # early tiles per half-class (chunk-0-only sources)
TL = TPB // 2 - TE     # late tiles per half-class (need the full table)
NE2 = 2 * TE * 128     # early idx per block (both half-classes share a gather)
NL = TL * 128          # late idx per half-class
SLOTH = TPH * 128      # slots per half
S16 = SLOTH // 16      # idx columns in packed [128, S16] layout
OWN = NBLK * PB        # own nodes per core (7168)
NID = NCORES * OWN     # internal id space (57344)
HALFR = NID // 2       # table half split (28672 < 32768)
D = 128                # table row width (bf16 -> 256B rows)
CH = 512               # dense chunk (nodes per matmul)
NCH_OWN = OWN // CH    # 14
NCH_ALL = NID // CH    # 112

F32 = mybir.dt.float32
BF16 = mybir.dt.bfloat16
FP8 = mybir.dt.float8e4
I16 = mybir.dt.int16
NP_BF16 = mybir.dt.np(BF16)
NP_FP8 = mybir.dt.np(FP8)

_cached = {}


def _build_bass(upto=99):
    nc = bacc.Bacc("TRN2", target_bir_lowering=False, debug=False,
                   num_devices=NCORES)

    # ---- I/O ----
    xT = nc.dram_tensor("xT", [IN, NID], BF16, kind="ExternalInput")
    xo = nc.dram_tensor("xo", [IN, OWN], BF16, kind="ExternalInput")
    # SAGE grouping: slots keyed by (dst block, src parity); idx = src//2 into
    # the 2-nodes-per-256B-row packed tables.
    idx_in = nc.dram_tensor("idx_in", [NBLK, 128, 2, S16], I16, kind="ExternalInput")
    dcode_in = nc.dram_tensor("dcode_in", [NBLK, 128, TPB], I16, kind="ExternalInput")
    iota_in = nc.dram_tensor("iota_in", [128, 128], I16, kind="ExternalInput")
    # GAT grouping: (src table half, early/late); 1-node-per-256B-row tables.
    idx_g_in = nc.dram_tensor("idx_g_in", [NBLK, 128, 128], I16, kind="ExternalInput")
    mask_g_in = nc.dram_tensor("mask_g_in", [NBLK, 128, TPB * 128], FP8, kind="ExternalInput")
    maskT_in = nc.dram_tensor("maskT_in", [NBLK, 128, TPB * 128], FP8, kind="ExternalInput")
    dgi_in = nc.dram_tensor("dgi_in", [NBLK, 128, 1], F32, kind="ExternalInput")

    wemb = nc.dram_tensor("wemb", [IN, H], BF16, kind="ExternalInput")
    bembr = nc.dram_tensor("bembr", [1, H], F32, kind="ExternalInput")
    bembc = nc.dram_tensor("bembc", [H, 1], F32, kind="ExternalInput")
    ws1 = nc.dram_tensor("ws1", [H, H], F32, kind="ExternalInput")
    wn1 = nc.dram_tensor("wn1", [H, H], F32, kind="ExternalInput")
    bn1 = nc.dram_tensor("bn1", [H, 1], F32, kind="ExternalInput")
    ws2 = nc.dram_tensor("ws2", [H, H], F32, kind="ExternalInput")
    wn2 = nc.dram_tensor("wn2", [H, H], F32, kind="ExternalInput")
    bn2 = nc.dram_tensor("bn2", [H, 1], F32, kind="ExternalInput")
    wl_in = nc.dram_tensor("wl_in", [H, HEADS], F32, kind="ExternalInput")
    wr_in = nc.dram_tensor("wr_in", [H, HEADS], F32, kind="ExternalInput")
    ulo_in = nc.dram_tensor("ulo_in", [128, H], BF16, kind="ExternalInput")
    uhi_in = nc.dram_tensor("uhi_in", [128, H], BF16, kind="ExternalInput")
    b1p = nc.dram_tensor("b1p", [H, 1], F32, kind="ExternalInput")
    w2_in = nc.dram_tensor("w2_in", [H, C], F32, kind="ExternalInput")
    b2c = nc.dram_tensor("b2c", [C, 1], F32, kind="ExternalInput")

    out = nc.dram_tensor("out", [OWN, C], F32, kind="ExternalOutput")

    with tile.TileContext(nc) as tc:
        with (
            tc.tile_pool(name="wpool", bufs=1) as wp,
            tc.tile_pool(name="sbuf", bufs=3) as sb,
            tc.tile_pool(name="big", bufs=1) as bigp,
            tc.tile_pool(name="psum", bufs=2, space="PSUM") as pp,
            tc.tile_pool(name="dram", bufs=1, space="DRAM") as dram,
        ):
            # ---- constants / weights resident in SBUF ----
            w_emb = wp.tile([IN, H], BF16)
            nc.sync.dma_start(w_emb[:], wemb[:])
            b_embr = wp.tile([1, H], F32)
            nc.sync.dma_start(b_embr[:], bembr[:])
            b_embc = wp.tile([H, 1], F32)
            nc.sync.dma_start(b_embc[:], bembc[:])
            w_s1 = wp.tile([H, H], F32); nc.sync.dma_start(w_s1[:], ws1[:])
            w_n1 = wp.tile([H, H], F32); nc.sync.dma_start(w_n1[:], wn1[:])
            b_n1 = wp.tile([H, 1], F32); nc.sync.dma_start(b_n1[:], bn1[:])
            w_s2 = wp.tile([H, H], F32); nc.sync.dma_start(w_s2[:], ws2[:])
            w_n2 = wp.tile([H, H], F32); nc.sync.dma_start(w_n2[:], wn2[:])
            b_n2 = wp.tile([H, 1], F32); nc.sync.dma_start(b_n2[:], bn2[:])
            w_l = wp.tile([H, HEADS], F32); nc.sync.dma_start(w_l[:], wl_in[:])
            w_r = wp.tile([H, HEADS], F32); nc.sync.dma_start(w_r[:], wr_in[:])
            u_lo = wp.tile([128, H], BF16); nc.sync.dma_start(u_lo[:], ulo_in[:])
            u_hi = wp.tile([128, H], BF16); nc.sync.dma_start(u_hi[:], uhi_in[:])
            b_1p = wp.tile([H, 1], F32); nc.sync.dma_start(b_1p[:], b1p[:])
            w_2 = wp.tile([H, C], F32); nc.sync.dma_start(w_2[:], w2_in[:])
            b_2 = wp.tile([C, 1], F32); nc.sync.dma_start(b_2[:], b2c[:])

            ones1 = wp.tile([1, 128], F32)
            nc.vector.memset(ones1[:], 1.0)
            iota_sb = wp.tile([128, 128], I16)
            nc.sync.dma_start(iota_sb[:], iota_in[:])
            epsz = wp.tile([1, HEADS * H + HEADS], F32)
            nc.vector.memset(epsz[:, 0:HEADS * H], 0.0)
            nc.vector.memset(epsz[:, HEADS * H:], 1e-20)
            id64f = wp.tile([64, 64], F32)
            make_identity(nc, id64f[:])
            id128f = wp.tile([128, 128], F32)
            make_identity(nc, id128f[:])
            id128b = wp.tile([128, 128], BF16)
            nc.vector.tensor_copy(id128b[:], id128f[:])
            id40f = wp.tile([40, 40], F32)
            make_identity(nc, id40f[:])

            # deginv: per-partition scalar per block -> SBUF [128, NBLK]
            dgi_sb = bigp.tile([128, NBLK], F32)
            nc.sync.dma_start(dgi_sb[:], dgi_in[:].rearrange("b p one -> p (b one)"))

            # persistent feature planes
            h1T = bigp.tile([H, OWN], F32, tag="hT", bufs=2)  # feat-major planes
            h2T = bigp.tile([H, OWN], F32, tag="hT", bufs=2)
            h3T = bigp.tile([H, OWN], F32, tag="hT", bufs=2)
            neighT = bigp.tile([H, OWN], F32)
            er_all = bigp.tile([128, NBLK, HEADS], BF16)
            og_nm = bigp.tile([128, NBLK, 2 * H * 2], BF16)  # node-major GAT out

            # DRAM tables. SAGE tables pack 2 nodes per 256B row, so the
            # compact [*, 64] AllGather output IS the gather table. The GAT
            # table needs 68 cols/node -> 256B rows + a repack after the AG.
            tab1 = dram.tile([HALFR, D], BF16)
            mine2 = dram.tile([OWN, 64], FP8)
            ag1a = dram.tile([NID // 2, 64], FP8, addr_space="Shared")
            ag1b = dram.tile([NID // 2, 64], FP8, addr_space="Shared")
            tab2q = dram.tile([NCORES, OWN // 2, 256], FP8)
            mineg = dram.tile([OWN, 72], FP8)
            agga = dram.tile([NID // 2, 72], FP8, addr_space="Shared")
            aggb = dram.tile([NID // 2, 72], FP8, addr_space="Shared")
            tabg_e = dram.tile([NID // 2, 256], FP8)
            tabga = dram.tile([4, OWN, 256], FP8)
            tabgb = dram.tile([4, OWN, 256], FP8)
            tab1v = tab1[:]

            # ================= P1: embed =================
            # full table (replicated): tab1 row r = bf16(h1 of nodes 2r, 2r+1)
            for ch2 in range(NCH_ALL // 2):
                xb = sb.tile([IN, 2 * CH], BF16, tag="xb")
                nc.gpsimd.dma_start(xb[:], xT[:, ch2 * 2 * CH:(ch2 + 1) * 2 * CH])
                stg = sb.tile([128, 8, H], BF16, tag="stg1")
                for sub in range(2):
                    pe = pp.tile([128, 4, H], F32, space="PSUM", tag="psA", bufs=4)
                    for q in range(4):
                        nc.tensor.matmul(
                            pe[:, q, :],
                            xb[:, sub * CH + q * 128:sub * CH + (q + 1) * 128],
                            w_emb[:], start=True, stop=False)
                        nc.tensor.matmul(pe[:, q, :], ones1[0:1, 0:128],
                                         b_embr[0:1, :], start=False, stop=True)
                    nc.vector.tensor_copy(stg[:, sub * 4:(sub + 1) * 4, :], pe[:])
                nc.scalar.dma_start(
                    tab1[ch2 * CH:(ch2 + 1) * CH, :].rearrange(
                        "r (two d) -> (r two) d", two=2).rearrange(
                        "(p q) d -> p q d", q=8), stg[:])
            # own features, feat-major (f32)
            for ch in range(NCH_OWN):
                xb2 = sb.tile([IN, CH], BF16, tag="xb")
                nc.gpsimd.dma_start(xb2[:], xo[:, ch * CH:(ch + 1) * CH])
                ph = pp.tile([H, CH], F32, space="PSUM", tag="psB", bufs=4)
                nc.tensor.matmul(ph[:], w_emb[:], xb2[:], start=True, stop=True)
                nc.scalar.activation(h1T[:, ch * CH:(ch + 1) * CH], ph[:],
                                     mybir.ActivationFunctionType.Identity,
                                     bias=b_embc[:], scale=1.0)

            # ============== SAGE layer helper ==============
            def sage_agg(table, chunk_cb=None):
                """Aggregate neighbor means into neighT (feat-major, f32).

                chunk_cb(ch) runs after each 4-block group's neighT is ready so
                the dense layer + row writes overlap the remaining gathers.
                Table is parity-packed: tiles 0:TPH hold even-src slots (cols
                0:H of the gathered rows), tiles TPH:TPB odd-src (cols H:2H).
                """
                gdt = table.dtype
                delem = 256 if gdt == FP8 else D
                it4 = None
                for b in range(NBLK):
                    if b % 4 == 0:
                        it4 = sb.tile([128, 4, 2, S16], I16, tag="it", bufs=2)
                        nc.sync.dma_start(it4[:], idx_in[b:b + 4].rearrange(
                            "q p h s -> p q h s"))
                    it = it4
                    dc = sb.tile([128, TPB], I16, tag="dc", bufs=4)
                    nc.sync.dma_start(dc[:], dcode_in[b])
                    mk = sb.tile([128, TPB * 128], FP8, tag="mk", bufs=4)
                    nc.vector.tensor_tensor(
                        mk[:].rearrange("p (t d) -> p t d", t=TPB),
                        dc[:].rearrange("p (t o) -> p t o", o=1).to_broadcast(
                            [128, TPB, 128]),
                        iota_sb[:].rearrange("p (o d) -> p o d", o=1).to_broadcast(
                            [128, TPB, 128]),
                        mybir.AluOpType.is_equal)
                    g = sb.tile([128, TPB, delem], gdt, tag="g", bufs=5)
                    nc.gpsimd.dma_gather(g[:, 0:TPH, :], table,
                                         it[:, b % 4, 0, :], SLOTH, SLOTH, delem)
                    nc.gpsimd.dma_gather(g[:, TPH:TPB, :], table,
                                         it[:, b % 4, 1, :], SLOTH, SLOTH, delem)
                    pa = pp.tile([128, H], F32, space="PSUM", tag="psA", bufs=4)
                    for t in range(TPB):
                        off = 0 if t < TPH else H
                        nc.tensor.matmul(pa[:], mk[:, t * 128:(t + 1) * 128],
                                         g[:, t, off:off + H],
                                         start=(t == 0), stop=(t == TPB - 1))
                    nb = sb.tile([128, H], F32, tag="nb")
                    nc.vector.tensor_scalar_mul(nb[:], pa[:], dgi_sb[:, b:b + 1])
                    pt = pp.tile([H, 128], F32, space="PSUM", tag="psA", bufs=4)
                    nc.tensor.transpose(pt[:], nb[:], id128f[:])
                    nc.vector.tensor_copy(neighT[:, b * 128:(b + 1) * 128], pt[:])
                    if chunk_cb is not None and b % 4 == 3:
                        chunk_cb(b // 4)

            def write_row_blk(hT_src, mine, width, b, with_el):
                """Transpose one block's feat-major features into rows."""
                ptr = pp.tile([128, H], F32, space="PSUM", tag="psA", bufs=4)
                nc.tensor.transpose(ptr[:], hT_src[:, b * 128:(b + 1) * 128],
                                    id64f[:])
                stg = sb.tile([128, width], mine.dtype, tag="stg2")
                nc.vector.tensor_copy(stg[:, 0:H], ptr[:])
                if with_el:
                    pel = pp.tile([128, HEADS], F32, space="PSUM", tag="psA", bufs=4)
                    nc.tensor.matmul(pel[:], hT_src[:, b * 128:(b + 1) * 128],
                                     w_l[:], start=True, stop=True)
                    nc.vector.tensor_copy(
                        stg[:, H:H + 2 * HEADS].bitcast(BF16), pel[:])
                    per = pp.tile([128, HEADS], F32, space="PSUM", tag="psA", bufs=4)
                    nc.tensor.matmul(per[:], hT_src[:, b * 128:(b + 1) * 128],
                                     w_r[:], start=True, stop=True)
                    nc.vector.tensor_copy(er_all[:, b, :], per[:])
                nc.sync.dma_start(mine[b * 128:(b + 1) * 128, :], stg[:])

            def ag_chunk(mine, agc, j):
                """AllGather chunk j: every rank's rows [j*OWN/2,(j+1)*OWN/2)
                concatenate into the contiguous chunk buffer."""
                nc.gpsimd.collective_compute(
                    "AllGather", mybir.AluOpType.bypass,
                    replica_groups=[list(range(NCORES))],
                    ins=[mine[j * OWN // 2:(j + 1) * OWN // 2, :].opt()],
                    outs=[agc[:].opt()],
                )

            def sage_fused_cb(hT_in, w_s, w_n, b_n, hT_out, mine, width,
                              with_el=False, on_c0=None):
                def cb(ch):
                    pd = pp.tile([H, CH], F32, space="PSUM", tag="psB", bufs=4)
                    nc.tensor.matmul(pd[:], w_s[:], hT_in[:, ch * CH:(ch + 1) * CH],
                                     start=True, stop=False)
                    nc.tensor.matmul(pd[:], w_n[:], neighT[:, ch * CH:(ch + 1) * CH],
                                     start=False, stop=True)
                    nc.scalar.activation(hT_out[:, ch * CH:(ch + 1) * CH], pd[:],
                                         mybir.ActivationFunctionType.Relu,
                                         bias=b_n[:], scale=1.0)
                    for b in range(ch * 4, ch * 4 + 4):
                        write_row_blk(hT_out, mine, width, b, with_el)
                    if on_c0 is not None and ch == NCH_OWN // 2 + 1:
                        on_c0()
                return cb

            def sage_agg_el(tabE, tabL, idx2, mask2, chunk_cb):
                """Early/late variant: tiles 0:TE p0-early, TE:2TE p1-early
                (sources living in AG chunk 0, read from tabE), then TL-tile
                late groups per parity from the full table tabL."""
                for b in range(NBLK):
                    it = sb.tile([128, 128], I16, tag="it2", bufs=3)
                    nc.sync.dma_start(it[:], idx2[b])
                    mk = sb.tile([128, TPB * 128], FP8, tag="mk", bufs=4)
                    nc.sync.dma_start(mk[:], mask2[b])
                    g = sb.tile([128, TPB, D], BF16, tag="g", bufs=5)
                    nc.gpsimd.dma_gather(g[:, 0:2 * TE, :], tabE,
                                         it[:, 0:NE2 // 16], NE2, NE2, D)
                    nc.gpsimd.dma_gather(g[:, 2 * TE:2 * TE + TL, :], tabL,
                                         it[:, NE2 // 16:(NE2 + NL) // 16], NL, NL, D)
                    nc.gpsimd.dma_gather(g[:, 2 * TE + TL:TPB, :], tabL,
                                         it[:, (NE2 + NL) // 16:(NE2 + 2 * NL) // 16],
                                         NL, NL, D)
                    pa = pp.tile([128, H], F32, space="PSUM", tag="psA", bufs=4)
                    for t in range(TPB):
                        off = 0 if (t < TE or 2 * TE <= t < 2 * TE + TL) else H
                        nc.tensor.matmul(pa[:], mk[:, t * 128:(t + 1) * 128],
                                         g[:, t, off:off + H],
                                         start=(t == 0), stop=(t == TPB - 1))
                    nb = sb.tile([128, H], F32, tag="nb")
                    nc.vector.tensor_scalar_mul(nb[:], pa[:], dgi_sb[:, b:b + 1])
                    pt = pp.tile([H, 128], F32, space="PSUM", tag="psA", bufs=4)
                    nc.tensor.transpose(pt[:], nb[:], id128f[:])
                    nc.vector.tensor_copy(neighT[:, b * 128:(b + 1) * 128], pt[:])
                    if chunk_cb is not None and b % 4 == 3:
                        chunk_cb(b // 4)

            # ================= SAGE 1 =================
            def sage1_c0():
                ag_chunk(mine2, ag1a, 0)
                nc.sync.dma_start(
                    tab2q[:, 0:OWN // 4, 0:128],
                    ag1a[:].rearrange("(c r two) d -> c r (two d)",
                                      c=NCORES, two=2))
            if upto >= 2:
                sage_agg(tab1v,
                         sage_fused_cb(h1T, w_s1, w_n1, b_n1, h2T, mine2, 64,
                                       on_c0=sage1_c0)
                         if upto >= 3 else None)
            if upto >= 3:
                ag_chunk(mine2, ag1b, 1)
                nc.sync.dma_start(
                    tab2q[:, OWN // 4:OWN // 2, 0:128],
                    ag1b[:].rearrange("(c r two) d -> c r (two d)",
                                      c=NCORES, two=2))

            # ================= SAGE 2 =================
            def sage2_c0():
                ag_chunk(mineg, agga, 0)
                # early GAT table: all chunk-0 rows (fits int16 indexing)
                nc.scalar.dma_start(tabg_e[:, 0:72], agga[:])
            if upto >= 4:
                sage_agg(tab2q[:].rearrange("c r d -> (c r) d"),
                         sage_fused_cb(h2T, w_s2, w_n2, b_n2, h3T, mineg, 72,
                                       with_el=True, on_c0=sage2_c0))
                ag_chunk(mineg, aggb, 1)
                # unified GAT tables from both chunks
                nc.sync.dma_start(
                    tabga[:, 0:OWN // 2, 0:72],
                    agga[0:HALFR // 2, :].rearrange("(c r) d -> c r d", c=4))
                nc.scalar.dma_start(
                    tabgb[:, 0:OWN // 2, 0:72],
                    agga[HALFR // 2:NID // 2, :].rearrange("(c r) d -> c r d", c=4))
                nc.sync.dma_start(
                    tabga[:, OWN // 2:OWN, 0:72],
                    aggb[0:HALFR // 2, :].rearrange("(c r) d -> c r d", c=4))
                nc.scalar.dma_start(
                    tabgb[:, OWN // 2:OWN, 0:72],
                    aggb[HALFR // 2:NID // 2, :].rearrange("(c r) d -> c r d", c=4))

            # ------- GAT dense + classifier (per 4-block chunk) -------
            def og_stage(ch, half):
                stgT = sb.tile([128, CH], BF16, tag=f"ogs{half}", bufs=2)
                for q in range(4):
                    b = ch * 4 + q
                    ptg = pp.tile([128, 128], BF16, space="PSUM", tag="psA", bufs=4)
                    nc.tensor.transpose(
                        ptg[:], og_nm[:, b, half * 128:(half + 1) * 128], id128b[:])
                    nc.vector.tensor_copy(stgT[:, q * 128:(q + 1) * 128], ptg[:])
                return stgT

            def gat_tail(ch):
                og_loS = og_stage(ch, 0)
                og_hiS = og_stage(ch, 1)
                p4 = pp.tile([H, CH], F32, space="PSUM", tag="psB", bufs=4)
                nc.tensor.matmul(p4[:], u_lo[:], og_loS[:],
                                 start=True, stop=False)
                nc.tensor.matmul(p4[:], u_hi[:], og_hiS[:],
                                 start=False, stop=True)
                h4 = sb.tile([H, CH], F32, tag="h4")
                nc.scalar.activation(h4[:], p4[:],
                                     mybir.ActivationFunctionType.Relu,
                                     bias=b_1p[:], scale=1.0)
                plg = pp.tile([C, CH], F32, space="PSUM", tag="psB", bufs=4)
                nc.tensor.matmul(plg[:], w_2[:], h4[:], start=True, stop=True)
                lg = sb.tile([C, CH], F32, tag="lg")
                nc.scalar.activation(lg[:], plg[:],
                                     mybir.ActivationFunctionType.Identity,
                                     bias=b_2[:], scale=1.0)
                ostg = sb.tile([128, 4, C], F32, tag="ostg")
                for q in range(4):
                    plt = pp.tile([128, C], F32, space="PSUM", tag="psA", bufs=4)
                    nc.tensor.transpose(plt[:], lg[:, q * 128:(q + 1) * 128], id40f[:])
                    nc.scalar.activation(ostg[:, q, :], plt[:],
                                         mybir.ActivationFunctionType.Identity)
                nc.sync.dma_start(
                    out[ch * CH:(ch + 1) * CH, :].rearrange("(q p) c -> p q c", p=128),
                    ostg[:])

            # ================= GAT aggregation =================
            _noW = _noER = _noPG = _noTR = False
            tabgaf = tabga[:].rearrange("c r d -> (c r) d")
            tabgbf = tabgb[:].rearrange("c r d -> (c r) d")
            for b in range(NBLK if upto >= 5 else 0):
                it4g = sb.tile([128, 128], I16, tag="it2", bufs=3)
                nc.sync.dma_start(it4g[:], idx_g_in[b])
                mk = sb.tile([128, TPB * 128], FP8, tag="mk", bufs=4)
                nc.sync.dma_start(mk[:], mask_g_in[b])
                mt = sb.tile([128, TPB * 128], FP8, tag="mt", bufs=3)
                nc.sync.dma_start(mt[:], maskT_in[b])
                g = sb.tile([128, TPB, 256], FP8, tag="g", bufs=5)
                nc.gpsimd.dma_gather(g[:, 0:2 * TE, :], tabg_e[:],
                                     it4g[:, 0:NE2 // 16], NE2, NE2, 256)
                nc.gpsimd.dma_gather(g[:, 2 * TE:2 * TE + TL, :], tabgaf,
                                     it4g[:, NE2 // 16:(NE2 + NL) // 16],
                                     NL, NL, 256)
                nc.gpsimd.dma_gather(g[:, 2 * TE + TL:TPB, :], tabgbf,
                                     it4g[:, (NE2 + NL) // 16:128],
                                     NL, NL, 256)
                # er broadcast to edge slots via maskT matmuls; the gathered
                # el values fold into the same PSUM group via an identity
                # matmul, so e = el + er exists in PSUM without a DVE add.
                perb = pp.tile([128, TPB, HEADS], F32, space="PSUM", tag="psA", bufs=4)
                for t in range(TPB):
                    nc.tensor.matmul(perb[:, t, :], mt[:, t * 128:(t + 1) * 128],
                                     er_all[:, b, :], start=True, stop=False)
                nc.tensor.matmul(perb[:], id128b[:],
                                 g[:, :, H:H + 2 * HEADS].bitcast(BF16),
                                 start=False, stop=True)
                eeL = sb.tile([128, TPB, HEADS], BF16, tag="et")
                nc.scalar.activation(eeL[:], perb[:],
                                     mybir.ActivationFunctionType.Prelu,
                                     alpha=SLOPE)
                wst = sb.tile([128, TPB, HEADS * H + HEADS], BF16, tag="wst", bufs=3)
                nc.scalar.activation(wst[:, :, HEADS * H:], eeL[:],
                                     mybir.ActivationFunctionType.Exp)
                pg = pp.tile([128, HEADS * H + HEADS], F32, space="PSUM", tag="psB", bufs=4)
                HT = TPB // 2
                for half in range(2):
                    tsl = slice(half * HT, (half + 1) * HT)
                    if _noW:
                        if b == 0 and half == 0:
                            nc.vector.memset(wst[:, :, 0:HEADS * H], 0.5)
                    else:
                        # fused per-head weighting: wst[p,t,h,f] = g[p,t,f]*ex[p,t,h]
                        nc.vector.tensor_mul(
                            wst[:, tsl, 0:HEADS * H].rearrange(
                                "p t (h f) -> p t h f", h=HEADS),
                            g[:, tsl, 0:H].rearrange(
                                "p t (o f) -> p t o f", o=1).to_broadcast(
                                    [128, HT, HEADS, H]),
                            wst[:, tsl, HEADS * H:].rearrange(
                                "p t (h o) -> p t h o", o=1).to_broadcast(
                                    [128, HT, HEADS, H]))
                    for t in range(half * HT, (half + 1) * HT):
                        nc.tensor.matmul(pg[:], mk[:, t * 128:(t + 1) * 128],
                                         wst[:, t, :], start=(t == 0),
                                         stop=False)
                nc.tensor.matmul(pg[:], ones1[0:1, 0:128],
                                 epsz[0:1, :], start=False, stop=True)
                # normalize by z and transpose for the dense phase
                zi = sb.tile([128, HEADS], F32, tag="zi")
                nc.vector.reciprocal(zi[:], pg[:, HEADS * H:])
                nc.vector.tensor_mul(
                    og_nm[:, b, :].rearrange("p (h f) -> p h f", h=HEADS),
                    pg[:, 0:HEADS * H].rearrange("p (h f) -> p h f", h=HEADS),
                    zi[:].to_broadcast([128, HEADS, H]))
            for ch in range(NCH_OWN if upto >= 6 else 0):
                gat_tail(ch)

            if upto < 6:
                zo = sb.tile([128, NBLK, C], F32, tag="zo")
                nc.vector.memset(zo[:], 0.0)
                nc.sync.dma_start(
                    out[:].rearrange("(q p) c -> p q c", p=128), zo[:])

    nc.compile()
    return nc


def _plan(src, dst):
    """Host-side graph partitioning. Returns per-core index/mask arrays."""
    src = np.asarray(src).astype(np.int64)
    dst = np.asarray(dst).astype(np.int64)

    def grouping(si, di, half, val):
        """Slot layout for one (half-assignment, idx-value) scheme."""
        gblk = di // PB
        grp = gblk * 2 + half
        cnt = np.bincount(grp, minlength=NCORES * NBLK * 2)
        if cnt.max() > SLOTH:
            return None
        order = np.lexsort((si, grp))
        g_sorted = grp[order]
        starts = np.zeros(NCORES * NBLK * 2 + 1, np.int64)
        np.cumsum(cnt, out=starts[1:])
        j_in_grp = np.arange(E, dtype=np.int64) - starts[g_sorted]
        e_di = di[order]
        e_half = half[order]
        e_gblk = gblk[order]
        e_core = e_gblk // NBLK
        e_blk = e_gblk % NBLK

        idx16 = np.zeros((NCORES, NBLK, 16, 2, S16), np.int16)
        idx16[e_core, e_blk, j_in_grp % 16, e_half, j_in_grp // 16] = \
            val[order].astype(np.int16)
        idx16 = np.broadcast_to(idx16[:, :, None, :, :, :],
                                (NCORES, NBLK, 8, 16, 2, S16)).reshape(
                                    NCORES, NBLK, 128, 2, S16).copy()

        t_of = (e_half * TPH + j_in_grp // 128).astype(np.int64)
        p_of = (j_in_grp % 128).astype(np.int64)
        d_of = (e_di % PB).astype(np.int64)
        # dst codes: one dst column (or 255 = empty slot) per (partition, tile)
        dcode = np.full((NCORES, NBLK, 128, TPB), 255, np.int16)
        dcode[e_core, e_blk, p_of, t_of] = d_of.astype(np.int16)
        return idx16, dcode

    def grouping_el(si, di, half, jcls, val_e, val_l, early_merged, with_maskT):
        """Early/late slot layout. Early tiles hold only chunk-0 (jcls==0)
        sources; overflow and all chunk-1 sources go to the late tiles."""
        CAP_E, CAP_L = TE * 128, TL * 128
        NG = NCORES * NBLK * 2
        gblk = di // PB
        key = gblk * 2 + half
        order = np.lexsort((si, jcls, key))
        k_s = key[order]
        cnt = np.bincount(key, minlength=NG)
        starts = np.zeros(NG + 1, np.int64)
        np.cumsum(cnt, out=starts[1:])
        r = np.arange(E, dtype=np.int64) - starts[k_s]
        j_s = jcls[order]
        j0cnt = np.bincount(key[jcls == 0], minlength=NG)
        etk = np.minimum(j0cnt, CAP_E)
        is_e = (j_s == 0) & (r < CAP_E)
        lr = r - etk[k_s]
        if int(np.where(is_e, 0, lr).max()) >= CAP_L:
            return None
        e_half = half[order]
        e_di = di[order]
        e_gblk = gblk[order]
        e_core = e_gblk // NBLK
        e_blk = e_gblk % NBLK

        tile = np.where(is_e, e_half * TE + r // 128,
                        2 * TE + e_half * TL + lr // 128)
        pos = np.where(is_e, r % 128, lr % 128)
        m8 = np.zeros((NCORES, NBLK, 128, TPB * 128), np.uint8)
        one_fp8 = np.array(1.0, NP_FP8).view(np.uint8).item()
        m8[e_core, e_blk, pos, tile * 128 + (e_di % PB)] = one_fp8
        mT8 = None
        if with_maskT:
            mT8 = m8.reshape(NCORES, NBLK, 128, TPB, 128).transpose(0, 1, 4, 3, 2)
            mT8 = np.ascontiguousarray(mT8).reshape(
                NCORES, NBLK, 128, TPB * 128).view(NP_FP8)

        # idx streams -> [.., 16ch, 128 cols] wrapped layout
        if early_merged:
            ej = e_half * CAP_E + r          # one gather, cols [0, 2*CAP_E/16)
            ecol, lcol0 = ej // 16, 2 * CAP_E // 16
            ech = ej % 16
        else:
            ecol = e_half * (CAP_E // 16) + r // 16
            ech = r % 16
            lcol0 = 2 * (CAP_E // 16)
        lj = lr
        col = np.where(is_e, ecol, lcol0 + e_half * (CAP_L // 16) + lj // 16)
        chan = np.where(is_e, ech, lj % 16)
        v = np.where(is_e, val_e[order], val_l[order]).astype(np.int16)
        idx16 = np.zeros((NCORES, NBLK, 16, 128), np.int16)
        idx16[e_core, e_blk, chan, col] = v
        idx16 = np.broadcast_to(idx16[:, :, None, :, :],
                                (NCORES, NBLK, 8, 16, 128)).reshape(
                                    NCORES, NBLK, 128, 128).copy()
        return idx16, m8.view(NP_FP8), mT8

    for seed in range(64):
        rng = np.random.default_rng(seed)
        perm = rng.permutation(NID)[:N].astype(np.int64)  # orig -> internal
        si = perm[src]
        di = perm[dst]
        gs = grouping(si, di, si % 2, si // 2)                  # SAGE: parity
        cc, kk = si // OWN, si % OWN
        jcls = (kk >= OWN // 2).astype(np.int64)
        half_g = (si >= HALFR).astype(np.int64)
        vEg = cc * (OWN // 2) + (kk % (OWN // 2))   # row in unified tabg_e
        vLg = si - half_g * HALFR
        gg = grouping_el(si, di, half_g, jcls, vEg, vLg, True, True)
        if gs is not None and gg is not None:
            break
    else:
        raise RuntimeError("could not pack edges into halves; increase NBLK")

    # deginv per dst slot
    deg = np.bincount(di, minlength=NID).astype(np.float32)
    dgi = (1.0 / np.maximum(deg, 1.0)).reshape(NCORES, NBLK, PB, 1)

    return perm, gs[0], gs[1], gg[0], gg[1], gg[2], dgi


def kernel(x, src, dst, W_embed, b_embed, Ws1, Wn1, bn1, Ws2, Wn2, bn2,
           Wg, al, ar, bg, W1, b1, W2, b2):
    x = np.asarray(x, np.float32)
    perm, idxS, dcS, idxG, mG, mTG, dgi = _plan(src, dst)

    if "nc" not in _cached:
        _cached["nc"] = _build_bass()
    nc = _cached["nc"]

    # weight preprocessing
    Wg = np.asarray(Wg, np.float32)
    al = np.asarray(al, np.float32)
    ar = np.asarray(ar, np.float32)
    W1 = np.asarray(W1, np.float32)
    WL = np.stack([Wg[:, h * H:(h + 1) * H] @ al[h] for h in range(HEADS)], 1)
    WR = np.stack([Wg[:, h * H:(h + 1) * H] @ ar[h] for h in range(HEADS)], 1)
    b1p = (np.asarray(b1, np.float32) + np.asarray(bg, np.float32) @ W1)
    U = [Wg[:, h * H:(h + 1) * H] @ W1[h * H:(h + 1) * H] for h in range(HEADS)]
    Ulo = np.vstack([U[0], U[1]]).astype(NP_BF16)
    Uhi = np.vstack([U[2], U[3]]).astype(NP_BF16)

    xT = np.zeros((IN, NID), np.float32)
    xT[:, perm] = x.T
    # embed writes table rows p-major (row = p*8+q within each 1024-chunk) so
    # the DMA emits 1KB descriptors; present xT columns in matching order
    xTs = np.ascontiguousarray(
        np.swapaxes(xT.reshape(IN, NCH_ALL // 2, 128, 8), 2, 3).reshape(IN, NID))

    common = {
        "xT": xTs.astype(NP_BF16),
        "wemb": np.asarray(W_embed, np.float32).astype(NP_BF16),
        "bembr": np.asarray(b_embed, np.float32).reshape(1, H),
        "bembc": np.asarray(b_embed, np.float32).reshape(H, 1),
        "ws1": np.asarray(Ws1, np.float32), "wn1": np.asarray(Wn1, np.float32),
        "bn1": np.asarray(bn1, np.float32).reshape(H, 1),
        "ws2": np.asarray(Ws2, np.float32), "wn2": np.asarray(Wn2, np.float32),
        "bn2": np.asarray(bn2, np.float32).reshape(H, 1),
        "wl_in": WL, "wr_in": WR,
        "ulo_in": Ulo, "uhi_in": Uhi,
        "b1p": b1p.reshape(H, 1),
        "w2_in": np.asarray(W2, np.float32),
        "b2c": np.asarray(b2, np.float32).reshape(C, 1),
        "iota_in": np.broadcast_to(np.arange(128, dtype=np.int16), (128, 128)).copy(),
    }
    in_maps = []
    for c in range(NCORES):
        m = dict(common)
        m["xo"] = np.ascontiguousarray(xT[:, c * OWN:(c + 1) * OWN]).astype(NP_BF16)
        m["idx_in"] = np.ascontiguousarray(idxS[c])
        m["dcode_in"] = np.ascontiguousarray(dcS[c])
        m["idx_g_in"] = np.ascontiguousarray(idxG[c])
        m["mask_g_in"] = np.ascontiguousarray(mG[c])
        m["maskT_in"] = np.ascontiguousarray(mTG[c])
        m["dgi_in"] = np.ascontiguousarray(dgi[c])
        in_maps.append(m)

    res = run_bass_kernel_spmd(nc, in_maps, core_ids=list(range(NCORES)))
    full = np.concatenate([res.results[c]["out"] for c in range(NCORES)], 0)
    return full[perm].astype(np.float32)



# revision 75
# speedup vs baseline: 1.0334x; 1.0334x over previous
"""Trainium2 Bass kernel for EnhancedGraphSAGE (embed -> 2x SAGE-mean -> GAT -> MLP).

Self-contained: takes full inputs, shards node-wise across 8 NeuronCores
internally, returns the full [N, C] output.

Design:
- Nodes are relabeled by a random permutation into NID = 8*56*128 internal ids
  (core-major, then 128-dst "blocks"). Each core owns its 56 blocks' dsts.
- Edges grouped by dst block; slots are padded to 128-wide tiles and expanded
  by dma_gather of 256B table rows, then aggregated per dst with TensorE
  matmuls against fp8 one-hot masks (lhsT = mask [128 slots, 128 dsts]).
- SAGE tables pack 2 nodes per 256B row (parity of src selects the 64-col
  window in the matmul), so idx = src//2 always fits int16 and the compact
  AllGather payload needs only a cheap local repack. SAGE masks are generated
  on-chip (DVE is_equal vs an iota) from 2-byte dst codes instead of loading
  14.7MB of one-hot masks per layer.
- AllGather payloads are fp8 (features; GAT also carries 4 bf16 el values in
  the 72B row), and each AG is split in two chunks: chunk 0 (each core's
  first half of rows) fires mid-aggregation of the previous layer so its
  transfer overlaps remaining gather work. GAT additionally keeps a unified
  "early" table of chunk-0 rows so early gather tiles can proceed before
  chunk 1 lands (tiles: 6 early + 5+5 late per block, grouped by src half
  for int16 range).
- GAT: softmax without max-subtraction (exp of leaky_relu bounded; leaky via
  ACT Prelu which shares the exp table set); er[dst] broadcast to edges via
  maskT matmul; z gets a 1e-20 floor via an extra PE accumulate row; per-head
  ex weighting on DVE; Wg folded into W1 on the host (U = Wg_h @ W1_h).
- Embed runs replicated (x in bf16, SWDGE-batched loads), writing the packed
  sage1 table directly.
"""

import numpy as np

import concourse.bacc as bacc
import concourse.bass as bass
import concourse.mybir as mybir
import concourse.tile as tile
from concourse.bass_utils import run_bass_kernel_spmd
from concourse.masks import make_identity

# Problem constants (hardcoded per spec)
N, E, IN, H, HEADS, C = 50000, 800000, 128, 64, 4, 40
SLOPE = 0.2

# Sharding geometry
NCORES = 8
NBLK = 56              # dst blocks per core
PB = 128               # dst slots per block
TPH = 8                # gather tiles per half (1024 idx limit of dma_gather)
TPB = 2 * TPH          # tiles per block
TE = 3                 # early tiles per half-class (chunk-0-only sources)
TL = TPB // 2 - TE     # late tiles per half-class (need the full table)
NE2 = 2 * TE * 128     # early idx per block (both half-classes share a gather)
NL = TL * 128          # late idx per half-class
SLOTH = TPH * 128      # slots per half
S16 = SLOTH // 16      # idx columns in packed [128, S16] layout
OWN = NBLK * PB        # own nodes per core (7168)
NID = NCORES * OWN     # internal id space (57344)
HALFR = NID // 2       # table half split (28672 < 32768)
D = 128                # table row width (bf16 -> 256B rows)
CH = 512               # dense chunk (nodes per matmul)
NCH_OWN = OWN // CH    # 14
NCH_ALL = NID // CH    # 112

F32 = mybir.dt.float32
BF16 = mybir.dt.bfloat16
FP8 = mybir.dt.float8e4
I16 = mybir.dt.int16
NP_BF16 = mybir.dt.np(BF16)
NP_FP8 = mybir.dt.np(FP8)

_cached = {}


def _build_bass(upto=99):
    nc = bacc.Bacc("TRN2", target_bir_lowering=False, debug=False,
                   num_devices=NCORES)

    # ---- I/O ----
    xT = nc.dram_tensor("xT", [IN, NID], BF16, kind="ExternalInput")
    xo = nc.dram_tensor("xo", [IN, OWN], BF16, kind="ExternalInput")
    # SAGE grouping: slots keyed by (dst block, src parity); idx = src//2 into
    # the 2-nodes-per-256B-row packed tables.
    idx_in = nc.dram_tensor("idx_in", [NBLK, 128, 2, S16], I16, kind="ExternalInput")
    dcode_in = nc.dram_tensor("dcode_in", [NBLK, 128, TPB], I16, kind="ExternalInput")
    iota_in = nc.dram_tensor("iota_in", [128, 128], I16, kind="ExternalInput")
    # GAT grouping: (src table half, early/late); 1-node-per-256B-row tables.
    idx_g_in = nc.dram_tensor("idx_g_in", [NBLK, 128, 128], I16, kind="ExternalInput")
    mask_g_in = nc.dram_tensor("mask_g_in", [NBLK, 128, TPB * 128], FP8, kind="ExternalInput")
    maskT_in = nc.dram_tensor("maskT_in", [NBLK, 128, TPB * 128], FP8, kind="ExternalInput")
    dgi_in = nc.dram_tensor("dgi_in", [NBLK, 128, 1], F32, kind="ExternalInput")

    wemb = nc.dram_tensor("wemb", [IN, H], BF16, kind="ExternalInput")
    bembr = nc.dram_tensor("bembr", [1, H], F32, kind="ExternalInput")
    bembc = nc.dram_tensor("bembc", [H, 1], F32, kind="ExternalInput")
    ws1 = nc.dram_tensor("ws1", [H, H], F32, kind="ExternalInput")
    wn1 = nc.dram_tensor("wn1", [H, H], F32, kind="ExternalInput")
    bn1 = nc.dram_tensor("bn1", [H, 1], F32, kind="ExternalInput")
    ws2 = nc.dram_tensor("ws2", [H, H], F32, kind="ExternalInput")
    wn2 = nc.dram_tensor("wn2", [H, H], F32, kind="ExternalInput")
    bn2 = nc.dram_tensor("bn2", [H, 1], F32, kind="ExternalInput")
    wl_in = nc.dram_tensor("wl_in", [H, HEADS], F32, kind="ExternalInput")
    wr_in = nc.dram_tensor("wr_in", [H, HEADS], F32, kind="ExternalInput")
    ulo_in = nc.dram_tensor("ulo_in", [128, H], BF16, kind="ExternalInput")
    uhi_in = nc.dram_tensor("uhi_in", [128, H], BF16, kind="ExternalInput")
    b1p = nc.dram_tensor("b1p", [H, 1], F32, kind="ExternalInput")
    w2_in = nc.dram_tensor("w2_in", [H, C], F32, kind="ExternalInput")
    b2c = nc.dram_tensor("b2c", [C, 1], F32, kind="ExternalInput")

    out = nc.dram_tensor("out", [OWN, C], F32, kind="ExternalOutput")

    with tile.TileContext(nc) as tc:
        with (
            tc.tile_pool(name="wpool", bufs=1) as wp,
            tc.tile_pool(name="sbuf", bufs=3) as sb,
            tc.tile_pool(name="big", bufs=1) as bigp,
            tc.tile_pool(name="psum", bufs=2, space="PSUM") as pp,
            tc.tile_pool(name="dram", bufs=1, space="DRAM") as dram,
        ):
            # ---- constants / weights resident in SBUF ----
            w_emb = wp.tile([IN, H], BF16)
            nc.sync.dma_start(w_emb[:], wemb[:])
            b_embr = wp.tile([1, H], F32)
            nc.sync.dma_start(b_embr[:], bembr[:])
            b_embc = wp.tile([H, 1], F32)
            nc.sync.dma_start(b_embc[:], bembc[:])
            w_s1 = wp.tile([H, H], F32); nc.sync.dma_start(w_s1[:], ws1[:])
            w_n1 = wp.tile([H, H], F32); nc.sync.dma_start(w_n1[:], wn1[:])
            b_n1 = wp.tile([H, 1], F32); nc.sync.dma_start(b_n1[:], bn1[:])
            w_s2 = wp.tile([H, H], F32); nc.sync.dma_start(w_s2[:], ws2[:])
            w_n2 = wp.tile([H, H], F32); nc.sync.dma_start(w_n2[:], wn2[:])
            b_n2 = wp.tile([H, 1], F32); nc.sync.dma_start(b_n2[:], bn2[:])
            w_l = wp.tile([H, HEADS], F32); nc.sync.dma_start(w_l[:], wl_in[:])
            w_r = wp.tile([H, HEADS], F32); nc.sync.dma_start(w_r[:], wr_in[:])
            u_lo = wp.tile([128, H], BF16); nc.sync.dma_start(u_lo[:], ulo_in[:])
            u_hi = wp.tile([128, H], BF16); nc.sync.dma_start(u_hi[:], uhi_in[:])
            b_1p = wp.tile([H, 1], F32); nc.sync.dma_start(b_1p[:], b1p[:])
            w_2 = wp.tile([H, C], F32); nc.sync.dma_start(w_2[:], w2_in[:])
            b_2 = wp.tile([C, 1], F32); nc.sync.dma_start(b_2[:], b2c[:])

            ones1 = wp.tile([1, 128], F32)
            nc.vector.memset(ones1[:], 1.0)
            iota_sb = wp.tile([128, 128], I16)
            nc.sync.dma_start(iota_sb[:], iota_in[:])
            epsz = wp.tile([1, HEADS * H + HEADS], F32)
            nc.vector.memset(epsz[:, 0:HEADS * H], 0.0)
            nc.vector.memset(epsz[:, HEADS * H:], 1e-20)
            id64f = wp.tile([64, 64], F32)
            make_identity(nc, id64f[:])
            id128f = wp.tile([128, 128], F32)
            make_identity(nc, id128f[:])
            id128b = wp.tile([128, 128], BF16)
            nc.vector.tensor_copy(id128b[:], id128f[:])
            id40f = wp.tile([40, 40], F32)
            make_identity(nc, id40f[:])

            # deginv: per-partition scalar per block -> SBUF [128, NBLK]
            dgi_sb = bigp.tile([128, NBLK], F32)
            nc.sync.dma_start(dgi_sb[:], dgi_in[:].rearrange("b p one -> p (b one)"))

            # persistent feature planes
            h1T = bigp.tile([H, OWN], F32, tag="hT", bufs=2)  # feat-major planes
            h2T = bigp.tile([H, OWN], F32, tag="hT", bufs=2)
            h3T = bigp.tile([H, OWN], F32, tag="hT", bufs=2)
            neighT = bigp.tile([H, OWN], F32)
            er_all = bigp.tile([128, NBLK, HEADS], BF16)
            og_nm = bigp.tile([128, NBLK, 2 * H * 2], BF16)  # node-major GAT out

            # DRAM tables. SAGE tables pack 2 nodes per 256B row, so the
            # compact [*, 64] AllGather output IS the gather table. The GAT
            # table needs 68 cols/node -> 256B rows + a repack after the AG.
            tab1 = dram.tile([HALFR, D], BF16)
            mine2 = dram.tile([OWN, 64], FP8)
            ag1a = dram.tile([NID // 2, 64], FP8, addr_space="Shared")
            ag1b = dram.tile([NID // 2, 64], FP8, addr_space="Shared")
            tab2q = dram.tile([NCORES, OWN // 2, 256], FP8)
            mineg = dram.tile([OWN, 72], FP8)
            agga = dram.tile([NID // 2, 72], FP8, addr_space="Shared")
            aggb = dram.tile([NID // 2, 72], FP8, addr_space="Shared")
            tabg_e = dram.tile([NID // 2, 256], FP8)
            tabga = dram.tile([4, OWN, 256], FP8)
            tabgb = dram.tile([4, OWN, 256], FP8)
            tab1v = tab1[:]

            # ================= P1: embed =================
            # full table (replicated): tab1 row r = bf16(h1 of nodes 2r, 2r+1)
            for ch2 in range(NCH_ALL // 2):
                xb = sb.tile([IN, 2 * CH], BF16, tag="xb")
                nc.gpsimd.dma_start(xb[:], xT[:, ch2 * 2 * CH:(ch2 + 1) * 2 * CH])
                stg = sb.tile([128, 8, H], BF16, tag="stg1")
                for sub in range(2):
                    pe = pp.tile([128, 4, H], F32, space="PSUM", tag="psA", bufs=4)
                    for q in range(4):
                        nc.tensor.matmul(
                            pe[:, q, :],
                            xb[:, sub * CH + q * 128:sub * CH + (q + 1) * 128],
                            w_emb[:], start=True, stop=False)
                        nc.tensor.matmul(pe[:, q, :], ones1[0:1, 0:128],
                                         b_embr[0:1, :], start=False, stop=True)
                    nc.vector.tensor_copy(stg[:, sub * 4:(sub + 1) * 4, :], pe[:])
                nc.scalar.dma_start(
                    tab1[ch2 * CH:(ch2 + 1) * CH, :].rearrange(
                        "r (two d) -> (r two) d", two=2).rearrange(
                        "(p q) d -> p q d", q=8), stg[:])
            # own features, feat-major (f32)
            for ch in range(NCH_OWN):
                xb2 = sb.tile([IN, CH], BF16, tag="xb")
                nc.gpsimd.dma_start(xb2[:], xo[:, ch * CH:(ch + 1) * CH])
                ph = pp.tile([H, CH], F32, space="PSUM", tag="psB", bufs=4)
                nc.tensor.matmul(ph[:], w_emb[:], xb2[:], start=True, stop=True)
                nc.scalar.activation(h1T[:, ch * CH:(ch + 1) * CH], ph[:],
                                     mybir.ActivationFunctionType.Identity,
                                     bias=b_embc[:], scale=1.0)

            # ============== SAGE layer helper ==============
            def sage_agg(table, chunk_cb=None):
                """Aggregate neighbor means into neighT (feat-major, f32).

                chunk_cb(ch) runs after each 4-block group's neighT is ready so
                the dense layer + row writes overlap the remaining gathers.
                Table is parity-packed: tiles 0:TPH hold even-src slots (cols
                0:H of the gathered rows), tiles TPH:TPB odd-src (cols H:2H).
                """
                gdt = table.dtype
                delem = 256 if gdt == FP8 else D
                it4 = None
                for b in range(NBLK):
                    if b % 4 == 0:
                        it4 = sb.tile([128, 4, 2, S16], I16, tag="it", bufs=2)
                        nc.sync.dma_start(it4[:], idx_in[b:b + 4].rearrange(
                            "q p h s -> p q h s"))
                    it = it4
                    dc = sb.tile([128, TPB], I16, tag="dc", bufs=4)
                    nc.sync.dma_start(dc[:], dcode_in[b])
                    mk = sb.tile([128, TPB * 128], FP8, tag="mk", bufs=4)
                    nc.vector.tensor_tensor(
                        mk[:].rearrange("p (t d) -> p t d", t=TPB),
                        dc[:].rearrange("p (t o) -> p t o", o=1).to_broadcast(
                            [128, TPB, 128]),
                        iota_sb[:].rearrange("p (o d) -> p o d", o=1).to_broadcast(
                            [128, TPB, 128]),
                        mybir.AluOpType.is_equal)
                    g = sb.tile([128, TPB, delem], gdt, tag="g", bufs=5)
                    nc.gpsimd.dma_gather(g[:, 0:TPH, :], table,
                                         it[:, b % 4, 0, :], SLOTH, SLOTH, delem)
                    nc.gpsimd.dma_gather(g[:, TPH:TPB, :], table,
                                         it[:, b % 4, 1, :], SLOTH, SLOTH, delem)
                    pa = pp.tile([128, H], F32, space="PSUM", tag="psA", bufs=4)
                    for t in range(TPB):
                        off = 0 if t < TPH else H
                        nc.tensor.matmul(pa[:], mk[:, t * 128:(t + 1) * 128],
                                         g[:, t, off:off + H],
                                         start=(t == 0), stop=(t == TPB - 1))
                    nb = sb.tile([128, H], F32, tag="nb")
                    nc.vector.tensor_scalar_mul(nb[:], pa[:], dgi_sb[:, b:b + 1])
                    pt = pp.tile([H, 128], F32, space="PSUM", tag="psA", bufs=4)
                    nc.tensor.transpose(pt[:], nb[:], id128f[:])
                    nc.vector.tensor_copy(neighT[:, b * 128:(b + 1) * 128], pt[:])
                    if chunk_cb is not None and b % 4 == 3:
                        chunk_cb(b // 4)

            def write_row_blk(hT_src, mine, width, b, with_el):
                """Transpose one block's feat-major features into rows."""
                ptr = pp.tile([128, H], F32, space="PSUM", tag="psA", bufs=4)
                nc.tensor.transpose(ptr[:], hT_src[:, b * 128:(b + 1) * 128],
                                    id64f[:])
                stg = sb.tile([128, width], mine.dtype, tag="stg2")
                nc.vector.tensor_copy(stg[:, 0:H], ptr[:])
                if with_el:
                    pel = pp.tile([128, HEADS], F32, space="PSUM", tag="psA", bufs=4)
                    nc.tensor.matmul(pel[:], hT_src[:, b * 128:(b + 1) * 128],
                                     w_l[:], start=True, stop=True)
                    nc.vector.tensor_copy(
                        stg[:, H:H + 2 * HEADS].bitcast(BF16), pel[:])
                    per = pp.tile([128, HEADS], F32, space="PSUM", tag="psA", bufs=4)
                    nc.tensor.matmul(per[:], hT_src[:, b * 128:(b + 1) * 128],
                                     w_r[:], start=True, stop=True)
                    nc.vector.tensor_copy(er_all[:, b, :], per[:])
                nc.sync.dma_start(mine[b * 128:(b + 1) * 128, :], stg[:])

            def ag_chunk(mine, agc, j):
                """AllGather chunk j: every rank's rows [j*OWN/2,(j+1)*OWN/2)
                concatenate into the contiguous chunk buffer."""
                nc.gpsimd.collective_compute(
                    "AllGather", mybir.AluOpType.bypass,
                    replica_groups=[list(range(NCORES))],
                    ins=[mine[j * OWN // 2:(j + 1) * OWN // 2, :].opt()],
                    outs=[agc[:].opt()],
                )

            def sage_fused_cb(hT_in, w_s, w_n, b_n, hT_out, mine, width,
                              with_el=False, on_c0=None):
                def cb(ch):
                    pd = pp.tile([H, CH], F32, space="PSUM", tag="psB", bufs=4)
                    nc.tensor.matmul(pd[:], w_s[:], hT_in[:, ch * CH:(ch + 1) * CH],
                                     start=True, stop=False)
                    nc.tensor.matmul(pd[:], w_n[:], neighT[:, ch * CH:(ch + 1) * CH],
                                     start=False, stop=True)
                    nc.scalar.activation(hT_out[:, ch * CH:(ch + 1) * CH], pd[:],
                                         mybir.ActivationFunctionType.Relu,
                                         bias=b_n[:], scale=1.0)
                    for b in range(ch * 4, ch * 4 + 4):
                        write_row_blk(hT_out, mine, width, b, with_el)
                    if on_c0 is not None and ch == NCH_OWN // 2 + 1:
                        on_c0()
                return cb

            def sage_agg_el(tabE, tabL, idx2, mask2, chunk_cb):
                """Early/late variant: tiles 0:TE p0-early, TE:2TE p1-early
                (sources living in AG chunk 0, read from tabE), then TL-tile
                late groups per parity from the full table tabL."""
                for b in range(NBLK):
                    it = sb.tile([128, 128], I16, tag="it2", bufs=3)
                    nc.sync.dma_start(it[:], idx2[b])
                    mk = sb.tile([128, TPB * 128], FP8, tag="mk", bufs=4)
                    nc.sync.dma_start(mk[:], mask2[b])
                    g = sb.tile([128, TPB, D], BF16, tag="g", bufs=5)
                    nc.gpsimd.dma_gather(g[:, 0:2 * TE, :], tabE,
                                         it[:, 0:NE2 // 16], NE2, NE2, D)
                    nc.gpsimd.dma_gather(g[:, 2 * TE:2 * TE + TL, :], tabL,
                                         it[:, NE2 // 16:(NE2 + NL) // 16], NL, NL, D)
                    nc.gpsimd.dma_gather(g[:, 2 * TE + TL:TPB, :], tabL,
                                         it[:, (NE2 + NL) // 16:(NE2 + 2 * NL) // 16],
                                         NL, NL, D)
                    pa = pp.tile([128, H], F32, space="PSUM", tag="psA", bufs=4)
                    for t in range(TPB):
                        off = 0 if (t < TE or 2 * TE <= t < 2 * TE + TL) else H
                        nc.tensor.matmul(pa[:], mk[:, t * 128:(t + 1) * 128],
                                         g[:, t, off:off + H],
                                         start=(t == 0), stop=(t == TPB - 1))
                    nb = sb.tile([128, H], F32, tag="nb")
                    nc.vector.tensor_scalar_mul(nb[:], pa[:], dgi_sb[:, b:b + 1])
                    pt = pp.tile([H, 128], F32, space="PSUM", tag="psA", bufs=4)
                    nc.tensor.transpose(pt[:], nb[:], id128f[:])
                    nc.vector.tensor_copy(neighT[:, b * 128:(b + 1) * 128], pt[:])
                    if chunk_cb is not None and b % 4 == 3:
                        chunk_cb(b // 4)

            # ================= SAGE 1 =================
            def sage1_c0():
                ag_chunk(mine2, ag1a, 0)
                nc.sync.dma_start(
                    tab2q[:, 0:OWN // 4, 0:128],
                    ag1a[:].rearrange("(c r two) d -> c r (two d)",
                                      c=NCORES, two=2))
            if upto >= 2:
                sage_agg(tab1v,
                         sage_fused_cb(h1T, w_s1, w_n1, b_n1, h2T, mine2, 64,
                                       on_c0=sage1_c0)
                         if upto >= 3 else None)
            if upto >= 3:
                ag_chunk(mine2, ag1b, 1)
                nc.sync.dma_start(
                    tab2q[:, OWN // 4:OWN // 2, 0:128],
                    ag1b[:].rearrange("(c r two) d -> c r (two d)",
                                      c=NCORES, two=2))

            # ================= SAGE 2 =================
            def sage2_c0():
                ag_chunk(mineg, agga, 0)
                # early GAT table: all chunk-0 rows (fits int16 indexing)
                nc.scalar.dma_start(tabg_e[:, 0:72], agga[:])
            if upto >= 4:
                sage_agg(tab2q[:].rearrange("c r d -> (c r) d"),
                         sage_fused_cb(h2T, w_s2, w_n2, b_n2, h3T, mineg, 72,
                                       with_el=True, on_c0=sage2_c0))
                ag_chunk(mineg, aggb, 1)
                # unified GAT tables from both chunks
                nc.sync.dma_start(
                    tabga[:, 0:OWN // 2, 0:72],
                    agga[0:HALFR // 2, :].rearrange("(c r) d -> c r d", c=4))
                nc.scalar.dma_start(
                    tabgb[:, 0:OWN // 2, 0:72],
                    agga[HALFR // 2:NID // 2, :].rearrange("(c r) d -> c r d", c=4))
                nc.sync.dma_start(
                    tabga[:, OWN // 2:OWN, 0:72],
                    aggb[0:HALFR // 2, :].rearrange("(c r) d -> c r d", c=4))
                nc.scalar.dma_start(
                    tabgb[:, OWN // 2:OWN, 0:72],
                    aggb[HALFR // 2:NID // 2, :].rearrange("(c r) d -> c r d", c=4))

            # ------- GAT dense + classifier (per 4-block chunk) -------
            def og_stage(ch, half):
                stgT = sb.tile([128, CH], BF16, tag=f"ogs{half}", bufs=2)
                for q in range(4):
                    b = ch * 4 + q
                    ptg = pp.tile([128, 128], BF16, space="PSUM", tag="psA", bufs=4)
                    nc.tensor.transpose(
                        ptg[:], og_nm[:, b, half * 128:(half + 1) * 128], id128b[:])
                    nc.vector.tensor_copy(stgT[:, q * 128:(q + 1) * 128], ptg[:])
                return stgT

            def gat_tail(ch):
                og_loS = og_stage(ch, 0)
                og_hiS = og_stage(ch, 1)
                p4 = pp.tile([H, CH], F32, space="PSUM", tag="psB", bufs=4)
                nc.tensor.matmul(p4[:], u_lo[:], og_loS[:],
                                 start=True, stop=False)
                nc.tensor.matmul(p4[:], u_hi[:], og_hiS[:],
                                 start=False, stop=True)
                h4 = sb.tile([H, CH], F32, tag="h4")
                nc.scalar.activation(h4[:], p4[:],
                                     mybir.ActivationFunctionType.Relu,
                                     bias=b_1p[:], scale=1.0)
                plg = pp.tile([C, CH], F32, space="PSUM", tag="psB", bufs=4)
                nc.tensor.matmul(plg[:], w_2[:], h4[:], start=True, stop=True)
                lg = sb.tile([C, CH], F32, tag="lg")
                nc.scalar.activation(lg[:], plg[:],
                                     mybir.ActivationFunctionType.Identity,
                                     bias=b_2[:], scale=1.0)
                ostg = sb.tile([128, 4, C], F32, tag="ostg")
                for q in range(4):
                    plt = pp.tile([128, C], F32, space="PSUM", tag="psA", bufs=4)
                    nc.tensor.transpose(plt[:], lg[:, q * 128:(q + 1) * 128], id40f[:])
                    nc.scalar.activation(ostg[:, q, :], plt[:],
                                         mybir.ActivationFunctionType.Identity)
                nc.sync.dma_start(
                    out[ch * CH:(ch + 1) * CH, :].rearrange("(q p) c -> p q c", p=128),
                    ostg[:])

            # ================= GAT aggregation =================
            _noW = _noER = _noPG = _noTR = False
            tabgaf = tabga[:].rearrange("c r d -> (c r) d")
            tabgbf = tabgb[:].rearrange("c r d -> (c r) d")
            for b in range(NBLK if upto >= 5 else 0):
                it4g = sb.tile([128, 128], I16, tag="it2", bufs=3)
                nc.sync.dma_start(it4g[:], idx_g_in[b])
                mk = sb.tile([128, TPB * 128], FP8, tag="mk", bufs=4)
                nc.sync.dma_start(mk[:], mask_g_in[b])
                mt = sb.tile([128, TPB * 128], FP8, tag="mt", bufs=3)
                nc.sync.dma_start(mt[:], maskT_in[b])
                g = sb.tile([128, TPB, 256], FP8, tag="g", bufs=5)
                nc.gpsimd.dma_gather(g[:, 0:2 * TE, :], tabg_e[:],
                                     it4g[:, 0:NE2 // 16], NE2, NE2, 256)
                nc.gpsimd.dma_gather(g[:, 2 * TE:2 * TE + TL, :], tabgaf,
                                     it4g[:, NE2 // 16:(NE2 + NL) // 16],
                                     NL, NL, 256)
                nc.gpsimd.dma_gather(g[:, 2 * TE + TL:TPB, :], tabgbf,
                                     it4g[:, (NE2 + NL) // 16:128],
                                     NL, NL, 256)
                # er broadcast to edge slots via maskT matmuls
                perb = pp.tile([128, TPB, HEADS], F32, space="PSUM", tag="psA", bufs=4)
                for t in range(TPB):
                    nc.tensor.matmul(perb[:, t, :], mt[:, t * 128:(t + 1) * 128],
                                     er_all[:, b, :], start=True, stop=True)
                # e = leaky_relu(el + er); ex = exp(e)  (no Lrelu table: max(x, .2x))
                ee = sb.tile([128, TPB, HEADS], BF16, tag="ee")
                nc.vector.tensor_add(ee[:], g[:, :, H:H + 2 * HEADS].bitcast(BF16), perb[:])
                eeL = sb.tile([128, TPB, HEADS], BF16, tag="et")
                nc.scalar.activation(eeL[:], ee[:],
                                     mybir.ActivationFunctionType.Prelu,
                                     alpha=SLOPE)
                wst = sb.tile([128, TPB, HEADS * H + HEADS], BF16, tag="wst", bufs=3)
                nc.scalar.activation(wst[:, :, HEADS * H:], eeL[:],
                                     mybir.ActivationFunctionType.Exp)
                pg = pp.tile([128, HEADS * H + HEADS], F32, space="PSUM", tag="psB", bufs=4)
                HT = TPB // 2
                for half in range(2):
                    tsl = slice(half * HT, (half + 1) * HT)
                    if _noW:
                        if b == 0 and half == 0:
                            nc.vector.memset(wst[:, :, 0:HEADS * H], 0.5)
                    else:
                        # fused per-head weighting: wst[p,t,h,f] = g[p,t,f]*ex[p,t,h]
                        nc.vector.tensor_mul(
                            wst[:, tsl, 0:HEADS * H].rearrange(
                                "p t (h f) -> p t h f", h=HEADS),
                            g[:, tsl, 0:H].rearrange(
                                "p t (o f) -> p t o f", o=1).to_broadcast(
                                    [128, HT, HEADS, H]),
                            wst[:, tsl, HEADS * H:].rearrange(
                                "p t (h o) -> p t h o", o=1).to_broadcast(
                                    [128, HT, HEADS, H]))
                    for t in range(half * HT, (half + 1) * HT):
                        nc.tensor.matmul(pg[:], mk[:, t * 128:(t + 1) * 128],
                                         wst[:, t, :], start=(t == 0),
                                         stop=False)
                nc.tensor.matmul(pg[:], ones1[0:1, 0:128],
                                 epsz[0:1, :], start=False, stop=True)
                # normalize by z and transpose for the dense phase
                zi = sb.tile([128, HEADS], F32, tag="zi")
                nc.vector.reciprocal(zi[:], pg[:, HEADS * H:])
                nc.vector.tensor_mul(
                    og_nm[:, b, :].rearrange("p (h f) -> p h f", h=HEADS),
                    pg[:, 0:HEADS * H].rearrange("p (h f) -> p h f", h=HEADS),
                    zi[:].to_broadcast([128, HEADS, H]))
            for ch in range(NCH_OWN if upto >= 6 else 0):
                gat_tail(ch)

            if upto < 6:
                zo = sb.tile([128, NBLK, C], F32, tag="zo")
                nc.vector.memset(zo[:], 0.0)
                nc.sync.dma_start(
                    out[:].rearrange("(q p) c -> p q c", p=128), zo[:])

    nc.compile()
    return nc


def _plan(src, dst):
    """Host-side graph partitioning. Returns per-core index/mask arrays."""
    src = np.asarray(src).astype(np.int64)
    dst = np.asarray(dst).astype(np.int64)

    def grouping(si, di, half, val):
        """Slot layout for one (half-assignment, idx-value) scheme."""
        gblk = di // PB
        grp = gblk * 2 + half
        cnt = np.bincount(grp, minlength=NCORES * NBLK * 2)
        if cnt.max() > SLOTH:
            return None
        order = np.lexsort((si, grp))
        g_sorted = grp[order]
        starts = np.zeros(NCORES * NBLK * 2 + 1, np.int64)
        np.cumsum(cnt, out=starts[1:])
        j_in_grp = np.arange(E, dtype=np.int64) - starts[g_sorted]
        e_di = di[order]
        e_half = half[order]
        e_gblk = gblk[order]
        e_core = e_gblk // NBLK
        e_blk = e_gblk % NBLK

        idx16 = np.zeros((NCORES, NBLK, 16, 2, S16), np.int16)
        idx16[e_core, e_blk, j_in_grp % 16, e_half, j_in_grp // 16] = \
            val[order].astype(np.int16)
        idx16 = np.broadcast_to(idx16[:, :, None, :, :, :],
                                (NCORES, NBLK, 8, 16, 2, S16)).reshape(
                                    NCORES, NBLK, 128, 2, S16).copy()

        t_of = (e_half * TPH + j_in_grp // 128).astype(np.int64)
        p_of = (j_in_grp % 128).astype(np.int64)
        d_of = (e_di % PB).astype(np.int64)
        # dst codes: one dst column (or 255 = empty slot) per (partition, tile)
        dcode = np.full((NCORES, NBLK, 128, TPB), 255, np.int16)
        dcode[e_core, e_blk, p_of, t_of] = d_of.astype(np.int16)
        return idx16, dcode

    def grouping_el(si, di, half, jcls, val_e, val_l, early_merged, with_maskT):
        """Early/late slot layout. Early tiles hold only chunk-0 (jcls==0)
        sources; overflow and all chunk-1 sources go to the late tiles."""
        CAP_E, CAP_L = TE * 128, TL * 128
        NG = NCORES * NBLK * 2
        gblk = di // PB
        key = gblk * 2 + half
        order = np.lexsort((si, jcls, key))
        k_s = key[order]
        cnt = np.bincount(key, minlength=NG)
        starts = np.zeros(NG + 1, np.int64)
        np.cumsum(cnt, out=starts[1:])
        r = np.arange(E, dtype=np.int64) - starts[k_s]
        j_s = jcls[order]
        j0cnt = np.bincount(key[jcls == 0], minlength=NG)
        etk = np.minimum(j0cnt, CAP_E)
        is_e = (j_s == 0) & (r < CAP_E)
        lr = r - etk[k_s]
        if int(np.where(is_e, 0, lr).max()) >= CAP_L:
            return None
        e_half = half[order]
        e_di = di[order]
        e_gblk = gblk[order]
        e_core = e_gblk // NBLK
        e_blk = e_gblk % NBLK

        tile = np.where(is_e, e_half * TE + r // 128,
                        2 * TE + e_half * TL + lr // 128)
        pos = np.where(is_e, r % 128, lr % 128)
        m8 = np.zeros((NCORES, NBLK, 128, TPB * 128), np.uint8)
        one_fp8 = np.array(1.0, NP_FP8).view(np.uint8).item()
        m8[e_core, e_blk, pos, tile * 128 + (e_di % PB)] = one_fp8
        mT8 = None
        if with_maskT:
            mT8 = m8.reshape(NCORES, NBLK, 128, TPB, 128).transpose(0, 1, 4, 3, 2)
            mT8 = np.ascontiguousarray(mT8).reshape(
                NCORES, NBLK, 128, TPB * 128).view(NP_FP8)

        # idx streams -> [.., 16ch, 128 cols] wrapped layout
        if early_merged:
            ej = e_half * CAP_E + r          # one gather, cols [0, 2*CAP_E/16)
            ecol, lcol0 = ej // 16, 2 * CAP_E // 16
            ech = ej % 16
        else:
            ecol = e_half * (CAP_E // 16) + r // 16
            ech = r % 16
            lcol0 = 2 * (CAP_E // 16)
        lj = lr
        col = np.where(is_e, ecol, lcol0 + e_half * (CAP_L // 16) + lj // 16)
        chan = np.where(is_e, ech, lj % 16)
        v = np.where(is_e, val_e[order], val_l[order]).astype(np.int16)
        idx16 = np.zeros((NCORES, NBLK, 16, 128), np.int16)
        idx16[e_core, e_blk, chan, col] = v
        idx16 = np.broadcast_to(idx16[:, :, None, :, :],
                                (NCORES, NBLK, 8, 16, 128)).reshape(
                                    NCORES, NBLK, 128, 128).copy()
        return idx16, m8.view(NP_FP8), mT8

    for seed in range(64):
        rng = np.random.default_rng(seed)
        perm = rng.permutation(NID)[:N].astype(np.int64)  # orig -> internal
        si = perm[src]
        di = perm[dst]
        gs = grouping(si, di, si % 2, si // 2)                  # SAGE: parity
        cc, kk = si // OWN, si % OWN
        jcls = (kk >= OWN // 2).astype(np.int64)
        half_g = (si >= HALFR).astype(np.int64)
        vEg = cc * (OWN // 2) + (kk % (OWN // 2))   # row in unified tabg_e
        vLg = si - half_g * HALFR
        gg = grouping_el(si, di, half_g, jcls, vEg, vLg, True, True)
        if gs is not None and gg is not None:
            break
    else:
        raise RuntimeError("could not pack edges into halves; increase NBLK")

    # deginv per dst slot
    deg = np.bincount(di, minlength=NID).astype(np.float32)
    dgi = (1.0 / np.maximum(deg, 1.0)).reshape(NCORES, NBLK, PB, 1)

    return perm, gs[0], gs[1], gg[0], gg[1], gg[2], dgi


def kernel(x, src, dst, W_embed, b_embed, Ws1, Wn1, bn1, Ws2, Wn2, bn2,
           Wg, al, ar, bg, W1, b1, W2, b2):
    x = np.asarray(x, np.float32)
    perm, idxS, dcS, idxG, mG, mTG, dgi = _plan(src, dst)

    if "nc" not in _cached:
        _cached["nc"] = _build_bass()
    nc = _cached["nc"]

    # weight preprocessing
    Wg = np.asarray(Wg, np.float32)
    al = np.asarray(al, np.float32)
    ar = np.asarray(ar, np.float32)
    W1 = np.asarray(W1, np.float32)
    WL = np.stack([Wg[:, h * H:(h + 1) * H] @ al[h] for h in range(HEADS)], 1)
    WR = np.stack([Wg[:, h * H:(h + 1) * H] @ ar[h] for h in range(HEADS)], 1)
    b1p = (np.asarray(b1, np.float32) + np.asarray(bg, np.float32) @ W1)
    U = [Wg[:, h * H:(h + 1) * H] @ W1[h * H:(h + 1) * H] for h in range(HEADS)]
    Ulo = np.vstack([U[0], U[1]]).astype(NP_BF16)
    Uhi = np.vstack([U[2], U[3]]).astype(NP_BF16)

    xT = np.zeros((IN, NID), np.float32)
    xT[:, perm] = x.T
    # embed writes table rows p-major (row = p*8+q within each 1024-chunk) so
    # the DMA emits 1KB descriptors; present xT columns in matching order
    xTs = np.ascontiguousarray(
        np.swapaxes(xT.reshape(IN, NCH_ALL // 2, 128, 8), 2, 3).reshape(IN, NID))

    common = {
        "xT": xTs.astype(NP_BF16),
        "wemb": np.asarray(W_embed, np.float32).astype(NP_BF16),
        "bembr": np.asarray(b_embed, np.float32).reshape(1, H),
        "bembc": np.asarray(b_embed, np.float32).reshape(H, 1),
        "ws1": np.asarray(Ws1, np.float32), "wn1": np.asarray(Wn1, np.float32),
        "bn1": np.asarray(bn1, np.float32).reshape(H, 1),
        "ws2": np.asarray(Ws2, np.float32), "wn2": np.asarray(Wn2, np.float32),
        "bn2": np.asarray(bn2, np.float32).reshape(H, 1),
        "wl_in": WL, "wr_in": WR,
        "ulo_in": Ulo, "uhi_in": Uhi,
        "b1p": b1p.reshape(H, 1),
        "w2_in": np.asarray(W2, np.float32),
        "b2c": np.asarray(b2, np.float32).reshape(C, 1),
        "iota_in": np.broadcast_to(np.arange(128, dtype=np.int16), (128, 128)).copy(),
    }
    in_maps = []
    for c in range(NCORES):
        m = dict(common)
        m["xo"] = np.ascontiguousarray(xT[:, c * OWN:(c + 1) * OWN]).astype(NP_BF16)
        m["idx_in"] = np.ascontiguousarray(idxS[c])
        m["dcode_in"] = np.ascontiguousarray(dcS[c])
        m["idx_g_in"] = np.ascontiguousarray(idxG[c])
        m["mask_g_in"] = np.ascontiguousarray(mG[c])
        m["maskT_in"] = np.ascontiguousarray(mTG[c])
        m["dgi_in"] = np.ascontiguousarray(dgi[c])
        in_maps.append(m)

    res = run_bass_kernel_spmd(nc, in_maps, core_ids=list(range(NCORES)))
    full = np.concatenate([res.results[c]["out"] for c in range(NCORES)], 0)
    return full[perm].astype(np.float32)



# revision 76
# speedup vs baseline: 1.0446x; 1.0109x over previous
"""Trainium2 Bass kernel for EnhancedGraphSAGE (embed -> 2x SAGE-mean -> GAT -> MLP).

Self-contained: takes full inputs, shards node-wise across 8 NeuronCores
internally, returns the full [N, C] output.

Design:
- Nodes are relabeled by a random permutation into NID = 8*56*128 internal ids
  (core-major, then 128-dst "blocks"). Each core owns its 56 blocks' dsts.
- Edges grouped by dst block; slots are padded to 128-wide tiles and expanded
  by dma_gather of 256B table rows, then aggregated per dst with TensorE
  matmuls against fp8 one-hot masks (lhsT = mask [128 slots, 128 dsts]).
- SAGE tables pack 2 nodes per 256B row (parity of src selects the 64-col
  window in the matmul), so idx = src//2 always fits int16 and the compact
  AllGather payload needs only a cheap local repack. SAGE masks are generated
  on-chip (DVE is_equal vs an iota) from 2-byte dst codes instead of loading
  14.7MB of one-hot masks per layer.
- AllGather payloads are fp8 (features; GAT also carries 4 bf16 el values in
  the 72B row), and each AG is split in two chunks: chunk 0 (each core's
  first half of rows) fires mid-aggregation of the previous layer so its
  transfer overlaps remaining gather work. GAT additionally keeps a unified
  "early" table of chunk-0 rows so early gather tiles can proceed before
  chunk 1 lands (tiles: 6 early + 5+5 late per block, grouped by src half
  for int16 range).
- GAT: softmax without max-subtraction (exp of leaky_relu bounded; leaky via
  ACT Prelu which shares the exp table set); er[dst] broadcast to edges via
  maskT matmul; z gets a 1e-20 floor via an extra PE accumulate row; per-head
  ex weighting on DVE; Wg folded into W1 on the host (U = Wg_h @ W1_h).
- Embed runs replicated (x in bf16, SWDGE-batched loads), writing the packed
  sage1 table directly.
"""

import numpy as np

import concourse.bacc as bacc
import concourse.bass as bass
import concourse.mybir as mybir
import concourse.tile as tile
from concourse.bass_utils import run_bass_kernel_spmd
from concourse.masks import make_identity

# Problem constants (hardcoded per spec)
N, E, IN, H, HEADS, C = 50000, 800000, 128, 64, 4, 40
SLOPE = 0.2

# Sharding geometry
NCORES = 8
NBLK = 56              # dst blocks per core
PB = 128               # dst slots per block
TPH = 8                # gather tiles per half (1024 idx limit of dma_gather)
TPB = 2 * TPH          # tiles per block
TE = 3                 # early tiles per half-class (chunk-0-only sources)
TL = TPB // 2 - TE     # late tiles per half-class (need the full table)
NE2 = 2 * TE * 128     # early idx per block (both half-classes share a gather)
NL = TL * 128          # late idx per half-class
SLOTH = TPH * 128      # slots per half
S16 = SLOTH // 16      # idx columns in packed [128, S16] layout
OWN = NBLK * PB        # own nodes per core (7168)
NID = NCORES * OWN     # internal id space (57344)
HALFR = NID // 2       # table half split (28672 < 32768)
D = 128                # table row width (bf16 -> 256B rows)
CH = 512               # dense chunk (nodes per matmul)
NCH_OWN = OWN // CH    # 14
NCH_ALL = NID // CH    # 112

F32 = mybir.dt.float32
BF16 = mybir.dt.bfloat16
FP8 = mybir.dt.float8e4
I16 = mybir.dt.int16
NP_BF16 = mybir.dt.np(BF16)
NP_FP8 = mybir.dt.np(FP8)

_cached = {}


def _build_bass(upto=99):
    nc = bacc.Bacc("TRN2", target_bir_lowering=False, debug=False,
                   num_devices=NCORES)

    # ---- I/O ----
    xT = nc.dram_tensor("xT", [IN, NID], BF16, kind="ExternalInput")
    xo = nc.dram_tensor("xo", [IN, OWN], BF16, kind="ExternalInput")
    # SAGE grouping: slots keyed by (dst block, src parity); idx = src//2 into
    # the 2-nodes-per-256B-row packed tables.
    idx_in = nc.dram_tensor("idx_in", [NBLK, 128, 2, S16], I16, kind="ExternalInput")
    dcode_in = nc.dram_tensor("dcode_in", [NBLK, 128, TPB], I16, kind="ExternalInput")
    iota_in = nc.dram_tensor("iota_in", [128, 128], I16, kind="ExternalInput")
    # GAT grouping: (src table half, early/late); 1-node-per-256B-row tables.
    idx_g_in = nc.dram_tensor("idx_g_in", [NBLK, 128, 128], I16, kind="ExternalInput")
    mask_g_in = nc.dram_tensor("mask_g_in", [NBLK, 128, TPB * 128], FP8, kind="ExternalInput")
    maskT_in = nc.dram_tensor("maskT_in", [NBLK, 128, TPB * 128], FP8, kind="ExternalInput")
    dgi_in = nc.dram_tensor("dgi_in", [NBLK, 128, 1], F32, kind="ExternalInput")

    wemb = nc.dram_tensor("wemb", [IN, H], BF16, kind="ExternalInput")
    bembr = nc.dram_tensor("bembr", [1, H], F32, kind="ExternalInput")
    bembc = nc.dram_tensor("bembc", [H, 1], F32, kind="ExternalInput")
    ws1 = nc.dram_tensor("ws1", [H, H], F32, kind="ExternalInput")
    wn1 = nc.dram_tensor("wn1", [H, H], F32, kind="ExternalInput")
    bn1 = nc.dram_tensor("bn1", [H, 1], F32, kind="ExternalInput")
    ws2 = nc.dram_tensor("ws2", [H, H], F32, kind="ExternalInput")
    wn2 = nc.dram_tensor("wn2", [H, H], F32, kind="ExternalInput")
    bn2 = nc.dram_tensor("bn2", [H, 1], F32, kind="ExternalInput")
    wl_in = nc.dram_tensor("wl_in", [H, HEADS], F32, kind="ExternalInput")
    wr_in = nc.dram_tensor("wr_in", [H, HEADS], F32, kind="ExternalInput")
    ulo_in = nc.dram_tensor("ulo_in", [128, H], BF16, kind="ExternalInput")
    uhi_in = nc.dram_tensor("uhi_in", [128, H], BF16, kind="ExternalInput")
    b1p = nc.dram_tensor("b1p", [H, 1], F32, kind="ExternalInput")
    w2_in = nc.dram_tensor("w2_in", [H, C], F32, kind="ExternalInput")
    b2c = nc.dram_tensor("b2c", [C, 1], F32, kind="ExternalInput")

    out = nc.dram_tensor("out", [OWN, C], F32, kind="ExternalOutput")

    with tile.TileContext(nc) as tc:
        with (
            tc.tile_pool(name="wpool", bufs=1) as wp,
            tc.tile_pool(name="sbuf", bufs=3) as sb,
            tc.tile_pool(name="big", bufs=1) as bigp,
            tc.tile_pool(name="psum", bufs=2, space="PSUM") as pp,
            tc.tile_pool(name="dram", bufs=1, space="DRAM") as dram,
        ):
            # ---- constants / weights resident in SBUF ----
            w_emb = wp.tile([IN, H], BF16)
            nc.sync.dma_start(w_emb[:], wemb[:])
            b_embr = wp.tile([1, H], F32)
            nc.sync.dma_start(b_embr[:], bembr[:])
            b_embc = wp.tile([H, 1], F32)
            nc.sync.dma_start(b_embc[:], bembc[:])
            w_s1 = wp.tile([H, H], F32); nc.sync.dma_start(w_s1[:], ws1[:])
            w_n1 = wp.tile([H, H], F32); nc.sync.dma_start(w_n1[:], wn1[:])
            b_n1 = wp.tile([H, 1], F32); nc.sync.dma_start(b_n1[:], bn1[:])
            w_s2 = wp.tile([H, H], F32); nc.sync.dma_start(w_s2[:], ws2[:])
            w_n2 = wp.tile([H, H], F32); nc.sync.dma_start(w_n2[:], wn2[:])
            b_n2 = wp.tile([H, 1], F32); nc.sync.dma_start(b_n2[:], bn2[:])
            w_lr = wp.tile([H, 2 * HEADS], F32)
            nc.sync.dma_start(w_lr[:, 0:HEADS], wl_in[:])
            nc.sync.dma_start(w_lr[:, HEADS:], wr_in[:])
            u_lo = wp.tile([128, H], BF16); nc.sync.dma_start(u_lo[:], ulo_in[:])
            u_hi = wp.tile([128, H], BF16); nc.sync.dma_start(u_hi[:], uhi_in[:])
            b_1p = wp.tile([H, 1], F32); nc.sync.dma_start(b_1p[:], b1p[:])
            w_2 = wp.tile([H, C], F32); nc.sync.dma_start(w_2[:], w2_in[:])
            b_2 = wp.tile([C, 1], F32); nc.sync.dma_start(b_2[:], b2c[:])

            ones1 = wp.tile([1, 128], F32)
            nc.vector.memset(ones1[:], 1.0)
            iota_sb = wp.tile([128, 128], I16)
            nc.sync.dma_start(iota_sb[:], iota_in[:])
            epsz = wp.tile([1, HEADS * H + HEADS], F32)
            nc.vector.memset(epsz[:, 0:HEADS * H], 0.0)
            nc.vector.memset(epsz[:, HEADS * H:], 1e-20)
            id64f = wp.tile([64, 64], F32)
            make_identity(nc, id64f[:])
            id128f = wp.tile([128, 128], F32)
            make_identity(nc, id128f[:])
            id128b = wp.tile([128, 128], BF16)
            nc.vector.tensor_copy(id128b[:], id128f[:])
            id40f = wp.tile([40, 40], F32)
            make_identity(nc, id40f[:])

            # deginv: per-partition scalar per block -> SBUF [128, NBLK]
            dgi_sb = bigp.tile([128, NBLK], F32)
            nc.sync.dma_start(dgi_sb[:], dgi_in[:].rearrange("b p one -> p (b one)"))

            # persistent feature planes
            h1T = bigp.tile([H, OWN], F32, tag="hT", bufs=2)  # feat-major planes
            h2T = bigp.tile([H, OWN], F32, tag="hT", bufs=2)
            h3T = bigp.tile([H, OWN], F32, tag="hT", bufs=2)
            neighT = bigp.tile([H, OWN], F32)
            er_all = bigp.tile([128, NBLK, HEADS], BF16)
            og_nm = bigp.tile([128, NBLK, 2 * H * 2], BF16)  # node-major GAT out

            # DRAM tables. SAGE tables pack 2 nodes per 256B row, so the
            # compact [*, 64] AllGather output IS the gather table. The GAT
            # table needs 68 cols/node -> 256B rows + a repack after the AG.
            tab1 = dram.tile([HALFR, D], BF16)
            mine2 = dram.tile([OWN, 64], FP8)
            ag1a = dram.tile([NID // 2, 64], FP8, addr_space="Shared")
            ag1b = dram.tile([NID // 2, 64], FP8, addr_space="Shared")
            tab2q = dram.tile([NCORES, OWN // 2, 256], FP8)
            mineg = dram.tile([OWN, 72], FP8)
            agga = dram.tile([NID // 2, 72], FP8, addr_space="Shared")
            aggb = dram.tile([NID // 2, 72], FP8, addr_space="Shared")
            tabg_e = dram.tile([NID // 2, 256], FP8)
            tabga = dram.tile([4, OWN, 256], FP8)
            tabgb = dram.tile([4, OWN, 256], FP8)
            tab1v = tab1[:]

            # ================= P1: embed =================
            # full table (replicated): tab1 row r = bf16(h1 of nodes 2r, 2r+1)
            for ch2 in range(NCH_ALL // 2):
                xb = sb.tile([IN, 2 * CH], BF16, tag="xb")
                nc.gpsimd.dma_start(xb[:], xT[:, ch2 * 2 * CH:(ch2 + 1) * 2 * CH])
                stg = sb.tile([128, 8, H], BF16, tag="stg1")
                for sub in range(2):
                    pe = pp.tile([128, 4, H], F32, space="PSUM", tag="psA", bufs=4)
                    for q in range(4):
                        nc.tensor.matmul(
                            pe[:, q, :],
                            xb[:, sub * CH + q * 128:sub * CH + (q + 1) * 128],
                            w_emb[:], start=True, stop=False)
                        nc.tensor.matmul(pe[:, q, :], ones1[0:1, 0:128],
                                         b_embr[0:1, :], start=False, stop=True)
                    nc.vector.tensor_copy(stg[:, sub * 4:(sub + 1) * 4, :], pe[:])
                nc.scalar.dma_start(
                    tab1[ch2 * CH:(ch2 + 1) * CH, :].rearrange(
                        "r (two d) -> (r two) d", two=2).rearrange(
                        "(p q) d -> p q d", q=8), stg[:])
            # own features, feat-major (f32)
            for ch in range(NCH_OWN):
                xb2 = sb.tile([IN, CH], BF16, tag="xb")
                nc.gpsimd.dma_start(xb2[:], xo[:, ch * CH:(ch + 1) * CH])
                ph = pp.tile([H, CH], F32, space="PSUM", tag="psB", bufs=4)
                nc.tensor.matmul(ph[:], w_emb[:], xb2[:], start=True, stop=True)
                nc.scalar.activation(h1T[:, ch * CH:(ch + 1) * CH], ph[:],
                                     mybir.ActivationFunctionType.Identity,
                                     bias=b_embc[:], scale=1.0)

            # ============== SAGE layer helper ==============
            def sage_agg(table, chunk_cb=None):
                """Aggregate neighbor means into neighT (feat-major, f32).

                chunk_cb(ch) runs after each 4-block group's neighT is ready so
                the dense layer + row writes overlap the remaining gathers.
                Table is parity-packed: tiles 0:TPH hold even-src slots (cols
                0:H of the gathered rows), tiles TPH:TPB odd-src (cols H:2H).
                """
                gdt = table.dtype
                delem = 256 if gdt == FP8 else D
                it4 = None
                for b in range(NBLK):
                    if b % 4 == 0:
                        it4 = sb.tile([128, 4, 2, S16], I16, tag="it", bufs=2)
                        nc.sync.dma_start(it4[:], idx_in[b:b + 4].rearrange(
                            "q p h s -> p q h s"))
                    it = it4
                    dc = sb.tile([128, TPB], I16, tag="dc", bufs=4)
                    nc.sync.dma_start(dc[:], dcode_in[b])
                    mk = sb.tile([128, TPB * 128], FP8, tag="mk", bufs=4)
                    nc.vector.tensor_tensor(
                        mk[:].rearrange("p (t d) -> p t d", t=TPB),
                        dc[:].rearrange("p (t o) -> p t o", o=1).to_broadcast(
                            [128, TPB, 128]),
                        iota_sb[:].rearrange("p (o d) -> p o d", o=1).to_broadcast(
                            [128, TPB, 128]),
                        mybir.AluOpType.is_equal)
                    g = sb.tile([128, TPB, delem], gdt, tag="g", bufs=5)
                    nc.gpsimd.dma_gather(g[:, 0:TPH, :], table,
                                         it[:, b % 4, 0, :], SLOTH, SLOTH, delem)
                    nc.gpsimd.dma_gather(g[:, TPH:TPB, :], table,
                                         it[:, b % 4, 1, :], SLOTH, SLOTH, delem)
                    pa = pp.tile([128, H], F32, space="PSUM", tag="psA", bufs=4)
                    for t in range(TPB):
                        off = 0 if t < TPH else H
                        nc.tensor.matmul(pa[:], mk[:, t * 128:(t + 1) * 128],
                                         g[:, t, off:off + H],
                                         start=(t == 0), stop=(t == TPB - 1))
                    nb = sb.tile([128, H], F32, tag="nb")
                    nc.vector.tensor_scalar_mul(nb[:], pa[:], dgi_sb[:, b:b + 1])
                    pt = pp.tile([H, 128], F32, space="PSUM", tag="psA", bufs=4)
                    nc.tensor.transpose(pt[:], nb[:], id128f[:])
                    nc.vector.tensor_copy(neighT[:, b * 128:(b + 1) * 128], pt[:])
                    if chunk_cb is not None and b % 4 == 3:
                        chunk_cb(b // 4)

            def write_row_blk(hT_src, stg4, q, b, with_el):
                """Transpose one block's feat-major features into rows."""
                ptr = pp.tile([128, H], F32, space="PSUM", tag="psA", bufs=4)
                nc.tensor.transpose(ptr[:], hT_src[:, b * 128:(b + 1) * 128],
                                    id64f[:])
                nc.vector.tensor_copy(stg4[:, q, 0:H], ptr[:])
                if with_el:
                    pel = pp.tile([128, 2 * HEADS], F32, space="PSUM", tag="psA",
                                  bufs=4)
                    nc.tensor.matmul(pel[:], hT_src[:, b * 128:(b + 1) * 128],
                                     w_lr[:], start=True, stop=True)
                    nc.vector.tensor_copy(
                        stg4[:, q, H:H + 2 * HEADS].bitcast(BF16),
                        pel[:, 0:HEADS])
                    nc.vector.tensor_copy(er_all[:, b, :], pel[:, HEADS:])

            def ag_chunk(mine, agc, j):
                """AllGather chunk j: every rank's rows [j*OWN/2,(j+1)*OWN/2)
                concatenate into the contiguous chunk buffer."""
                nc.gpsimd.collective_compute(
                    "AllGather", mybir.AluOpType.bypass,
                    replica_groups=[list(range(NCORES))],
                    ins=[mine[j * OWN // 2:(j + 1) * OWN // 2, :].opt()],
                    outs=[agc[:].opt()],
                )

            def sage_fused_cb(hT_in, w_s, w_n, b_n, hT_out, mine, width,
                              with_el=False, on_c0=None):
                def cb(ch):
                    pd = pp.tile([H, CH], F32, space="PSUM", tag="psB", bufs=4)
                    nc.tensor.matmul(pd[:], w_s[:], hT_in[:, ch * CH:(ch + 1) * CH],
                                     start=True, stop=False)
                    nc.tensor.matmul(pd[:], w_n[:], neighT[:, ch * CH:(ch + 1) * CH],
                                     start=False, stop=True)
                    nc.scalar.activation(hT_out[:, ch * CH:(ch + 1) * CH], pd[:],
                                         mybir.ActivationFunctionType.Relu,
                                         bias=b_n[:], scale=1.0)
                    stg4 = sb.tile([128, 4, width], mine.dtype, tag="stg2",
                                   bufs=2)
                    for q in range(4):
                        write_row_blk(hT_out, stg4, q, ch * 4 + q, with_el)
                    nc.sync.dma_start(
                        mine[ch * CH:(ch + 1) * CH, :].rearrange(
                            "(q p) w -> p q w", p=128), stg4[:])
                    if on_c0 is not None and ch == NCH_OWN // 2 + 1:
                        on_c0()
                return cb

            def sage_agg_el(tabE, tabL, idx2, mask2, chunk_cb):
                """Early/late variant: tiles 0:TE p0-early, TE:2TE p1-early
                (sources living in AG chunk 0, read from tabE), then TL-tile
                late groups per parity from the full table tabL."""
                for b in range(NBLK):
                    it = sb.tile([128, 128], I16, tag="it2", bufs=3)
                    nc.sync.dma_start(it[:], idx2[b])
                    mk = sb.tile([128, TPB * 128], FP8, tag="mk", bufs=4)
                    nc.sync.dma_start(mk[:], mask2[b])
                    g = sb.tile([128, TPB, D], BF16, tag="g", bufs=5)
                    nc.gpsimd.dma_gather(g[:, 0:2 * TE, :], tabE,
                                         it[:, 0:NE2 // 16], NE2, NE2, D)
                    nc.gpsimd.dma_gather(g[:, 2 * TE:2 * TE + TL, :], tabL,
                                         it[:, NE2 // 16:(NE2 + NL) // 16], NL, NL, D)
                    nc.gpsimd.dma_gather(g[:, 2 * TE + TL:TPB, :], tabL,
                                         it[:, (NE2 + NL) // 16:(NE2 + 2 * NL) // 16],
                                         NL, NL, D)
                    pa = pp.tile([128, H], F32, space="PSUM", tag="psA", bufs=4)
                    for t in range(TPB):
                        off = 0 if (t < TE or 2 * TE <= t < 2 * TE + TL) else H
                        nc.tensor.matmul(pa[:], mk[:, t * 128:(t + 1) * 128],
                                         g[:, t, off:off + H],
                                         start=(t == 0), stop=(t == TPB - 1))
                    nb = sb.tile([128, H], F32, tag="nb")
                    nc.vector.tensor_scalar_mul(nb[:], pa[:], dgi_sb[:, b:b + 1])
                    pt = pp.tile([H, 128], F32, space="PSUM", tag="psA", bufs=4)
                    nc.tensor.transpose(pt[:], nb[:], id128f[:])
                    nc.vector.tensor_copy(neighT[:, b * 128:(b + 1) * 128], pt[:])
                    if chunk_cb is not None and b % 4 == 3:
                        chunk_cb(b // 4)

            # ================= SAGE 1 =================
            def sage1_c0():
                ag_chunk(mine2, ag1a, 0)
                nc.sync.dma_start(
                    tab2q[:, 0:OWN // 4, 0:128],
                    ag1a[:].rearrange("(c r two) d -> c r (two d)",
                                      c=NCORES, two=2))
            if upto >= 2:
                sage_agg(tab1v,
                         sage_fused_cb(h1T, w_s1, w_n1, b_n1, h2T, mine2, 64,
                                       on_c0=sage1_c0)
                         if upto >= 3 else None)
            if upto >= 3:
                ag_chunk(mine2, ag1b, 1)
                nc.sync.dma_start(
                    tab2q[:, OWN // 4:OWN // 2, 0:128],
                    ag1b[:].rearrange("(c r two) d -> c r (two d)",
                                      c=NCORES, two=2))

            # ================= SAGE 2 =================
            def sage2_c0():
                ag_chunk(mineg, agga, 0)
                # early GAT table: all chunk-0 rows (fits int16 indexing)
                nc.scalar.dma_start(tabg_e[:, 0:72], agga[:])
            if upto >= 4:
                sage_agg(tab2q[:].rearrange("c r d -> (c r) d"),
                         sage_fused_cb(h2T, w_s2, w_n2, b_n2, h3T, mineg, 72,
                                       with_el=True, on_c0=sage2_c0))
                ag_chunk(mineg, aggb, 1)
                # unified GAT tables from both chunks
                nc.sync.dma_start(
                    tabga[:, 0:OWN // 2, 0:72],
                    agga[0:HALFR // 2, :].rearrange("(c r) d -> c r d", c=4))
                nc.scalar.dma_start(
                    tabgb[:, 0:OWN // 2, 0:72],
                    agga[HALFR // 2:NID // 2, :].rearrange("(c r) d -> c r d", c=4))
                nc.sync.dma_start(
                    tabga[:, OWN // 2:OWN, 0:72],
                    aggb[0:HALFR // 2, :].rearrange("(c r) d -> c r d", c=4))
                nc.scalar.dma_start(
                    tabgb[:, OWN // 2:OWN, 0:72],
                    aggb[HALFR // 2:NID // 2, :].rearrange("(c r) d -> c r d", c=4))

            # ------- GAT dense + classifier (per 4-block chunk) -------
            def og_stage(ch, half):
                stgT = sb.tile([128, CH], BF16, tag=f"ogs{half}", bufs=2)
                for q in range(4):
                    b = ch * 4 + q
                    ptg = pp.tile([128, 128], BF16, space="PSUM", tag="psA", bufs=4)
                    nc.tensor.transpose(
                        ptg[:], og_nm[:, b, half * 128:(half + 1) * 128], id128b[:])
                    nc.vector.tensor_copy(stgT[:, q * 128:(q + 1) * 128], ptg[:])
                return stgT

            def gat_tail(ch):
                og_loS = og_stage(ch, 0)
                og_hiS = og_stage(ch, 1)
                p4 = pp.tile([H, CH], F32, space="PSUM", tag="psB", bufs=4)
                nc.tensor.matmul(p4[:], u_lo[:], og_loS[:],
                                 start=True, stop=False)
                nc.tensor.matmul(p4[:], u_hi[:], og_hiS[:],
                                 start=False, stop=True)
                h4 = sb.tile([H, CH], F32, tag="h4")
                nc.scalar.activation(h4[:], p4[:],
                                     mybir.ActivationFunctionType.Relu,
                                     bias=b_1p[:], scale=1.0)
                plg = pp.tile([C, CH], F32, space="PSUM", tag="psB", bufs=4)
                nc.tensor.matmul(plg[:], w_2[:], h4[:], start=True, stop=True)
                lg = sb.tile([C, CH], F32, tag="lg")
                nc.scalar.activation(lg[:], plg[:],
                                     mybir.ActivationFunctionType.Identity,
                                     bias=b_2[:], scale=1.0)
                ostg = sb.tile([128, 4, C], F32, tag="ostg")
                for q in range(4):
                    plt = pp.tile([128, C], F32, space="PSUM", tag="psA", bufs=4)
                    nc.tensor.transpose(plt[:], lg[:, q * 128:(q + 1) * 128], id40f[:])
                    nc.scalar.activation(ostg[:, q, :], plt[:],
                                         mybir.ActivationFunctionType.Identity)
                nc.sync.dma_start(
                    out[ch * CH:(ch + 1) * CH, :].rearrange("(q p) c -> p q c", p=128),
                    ostg[:])

            # ================= GAT aggregation =================
            _noW = _noER = _noPG = _noTR = False
            tabgaf = tabga[:].rearrange("c r d -> (c r) d")
            tabgbf = tabgb[:].rearrange("c r d -> (c r) d")
            for b in range(NBLK if upto >= 5 else 0):
                it4g = sb.tile([128, 128], I16, tag="it2", bufs=3)
                nc.sync.dma_start(it4g[:], idx_g_in[b])
                mk = sb.tile([128, TPB * 128], FP8, tag="mk", bufs=4)
                nc.sync.dma_start(mk[:], mask_g_in[b])
                mt = sb.tile([128, TPB * 128], FP8, tag="mt", bufs=3)
                nc.sync.dma_start(mt[:], maskT_in[b])
                g = sb.tile([128, TPB, 256], FP8, tag="g", bufs=5)
                nc.gpsimd.dma_gather(g[:, 0:2 * TE, :], tabg_e[:],
                                     it4g[:, 0:NE2 // 16], NE2, NE2, 256)
                nc.gpsimd.dma_gather(g[:, 2 * TE:2 * TE + TL, :], tabgaf,
                                     it4g[:, NE2 // 16:(NE2 + NL) // 16],
                                     NL, NL, 256)
                nc.gpsimd.dma_gather(g[:, 2 * TE + TL:TPB, :], tabgbf,
                                     it4g[:, (NE2 + NL) // 16:128],
                                     NL, NL, 256)
                # er broadcast to edge slots via maskT matmuls
                perb = pp.tile([128, TPB, HEADS], F32, space="PSUM", tag="psA", bufs=4)
                for t in range(TPB):
                    nc.tensor.matmul(perb[:, t, :], mt[:, t * 128:(t + 1) * 128],
                                     er_all[:, b, :], start=True, stop=True)
                # e = leaky_relu(el + er); ex = exp(e)  (no Lrelu table: max(x, .2x))
                ee = sb.tile([128, TPB, HEADS], BF16, tag="ee")
                nc.vector.tensor_add(ee[:], g[:, :, H:H + 2 * HEADS].bitcast(BF16), perb[:])
                eeL = sb.tile([128, TPB, HEADS], BF16, tag="et")
                nc.scalar.activation(eeL[:], ee[:],
                                     mybir.ActivationFunctionType.Prelu,
                                     alpha=SLOPE)
                wst = sb.tile([128, TPB, HEADS * H + HEADS], BF16, tag="wst", bufs=3)
                nc.scalar.activation(wst[:, :, HEADS * H:], eeL[:],
                                     mybir.ActivationFunctionType.Exp)
                pg = pp.tile([128, HEADS * H + HEADS], F32, space="PSUM", tag="psB", bufs=4)
                HT = TPB // 2
                for half in range(2):
                    tsl = slice(half * HT, (half + 1) * HT)
                    if _noW:
                        if b == 0 and half == 0:
                            nc.vector.memset(wst[:, :, 0:HEADS * H], 0.5)
                    else:
                        # fused per-head weighting: wst[p,t,h,f] = g[p,t,f]*ex[p,t,h]
                        nc.vector.tensor_mul(
                            wst[:, tsl, 0:HEADS * H].rearrange(
                                "p t (h f) -> p t h f", h=HEADS),
                            g[:, tsl, 0:H].rearrange(
                                "p t (o f) -> p t o f", o=1).to_broadcast(
                                    [128, HT, HEADS, H]),
                            wst[:, tsl, HEADS * H:].rearrange(
                                "p t (h o) -> p t h o", o=1).to_broadcast(
                                    [128, HT, HEADS, H]))
                    for t in range(half * HT, (half + 1) * HT):
                        nc.tensor.matmul(pg[:], mk[:, t * 128:(t + 1) * 128],
                                         wst[:, t, :], start=(t == 0),
                                         stop=False)
                nc.tensor.matmul(pg[:], ones1[0:1, 0:128],
                                 epsz[0:1, :], start=False, stop=True)
                # normalize by z and transpose for the dense phase
                zi = sb.tile([128, HEADS], F32, tag="zi")
                nc.vector.reciprocal(zi[:], pg[:, HEADS * H:])
                nc.vector.tensor_mul(
                    og_nm[:, b, :].rearrange("p (h f) -> p h f", h=HEADS),
                    pg[:, 0:HEADS * H].rearrange("p (h f) -> p h f", h=HEADS),
                    zi[:].to_broadcast([128, HEADS, H]))
            for ch in range(NCH_OWN if upto >= 6 else 0):
                gat_tail(ch)

            if upto < 6:
                zo = sb.tile([128, NBLK, C], F32, tag="zo")
                nc.vector.memset(zo[:], 0.0)
                nc.sync.dma_start(
                    out[:].rearrange("(q p) c -> p q c", p=128), zo[:])

    nc.compile()
    return nc


def _plan(src, dst):
    """Host-side graph partitioning. Returns per-core index/mask arrays."""
    src = np.asarray(src).astype(np.int64)
    dst = np.asarray(dst).astype(np.int64)

    def grouping(si, di, half, val):
        """Slot layout for one (half-assignment, idx-value) scheme."""
        gblk = di // PB
        grp = gblk * 2 + half
        cnt = np.bincount(grp, minlength=NCORES * NBLK * 2)
        if cnt.max() > SLOTH:
            return None
        order = np.lexsort((si, grp))
        g_sorted = grp[order]
        starts = np.zeros(NCORES * NBLK * 2 + 1, np.int64)
        np.cumsum(cnt, out=starts[1:])
        j_in_grp = np.arange(E, dtype=np.int64) - starts[g_sorted]
        e_di = di[order]
        e_half = half[order]
        e_gblk = gblk[order]
        e_core = e_gblk // NBLK
        e_blk = e_gblk % NBLK

        idx16 = np.zeros((NCORES, NBLK, 16, 2, S16), np.int16)
        idx16[e_core, e_blk, j_in_grp % 16, e_half, j_in_grp // 16] = \
            val[order].astype(np.int16)
        idx16 = np.broadcast_to(idx16[:, :, None, :, :, :],
                                (NCORES, NBLK, 8, 16, 2, S16)).reshape(
                                    NCORES, NBLK, 128, 2, S16).copy()

        t_of = (e_half * TPH + j_in_grp // 128).astype(np.int64)
        p_of = (j_in_grp % 128).astype(np.int64)
        d_of = (e_di % PB).astype(np.int64)
        # dst codes: one dst column (or 255 = empty slot) per (partition, tile)
        dcode = np.full((NCORES, NBLK, 128, TPB), 255, np.int16)
        dcode[e_core, e_blk, p_of, t_of] = d_of.astype(np.int16)
        return idx16, dcode

    def grouping_el(si, di, half, jcls, val_e, val_l, early_merged, with_maskT):
        """Early/late slot layout. Early tiles hold only chunk-0 (jcls==0)
        sources; overflow and all chunk-1 sources go to the late tiles."""
        CAP_E, CAP_L = TE * 128, TL * 128
        NG = NCORES * NBLK * 2
        gblk = di // PB
        key = gblk * 2 + half
        order = np.lexsort((si, jcls, key))
        k_s = key[order]
        cnt = np.bincount(key, minlength=NG)
        starts = np.zeros(NG + 1, np.int64)
        np.cumsum(cnt, out=starts[1:])
        r = np.arange(E, dtype=np.int64) - starts[k_s]
        j_s = jcls[order]
        j0cnt = np.bincount(key[jcls == 0], minlength=NG)
        etk = np.minimum(j0cnt, CAP_E)
        is_e = (j_s == 0) & (r < CAP_E)
        lr = r - etk[k_s]
        if int(np.where(is_e, 0, lr).max()) >= CAP_L:
            return None
        e_half = half[order]
        e_di = di[order]
        e_gblk = gblk[order]
        e_core = e_gblk // NBLK
        e_blk = e_gblk % NBLK

        tile = np.where(is_e, e_half * TE + r // 128,
                        2 * TE + e_half * TL + lr // 128)
        pos = np.where(is_e, r % 128, lr % 128)
        m8 = np.zeros((NCORES, NBLK, 128, TPB * 128), np.uint8)
        one_fp8 = np.array(1.0, NP_FP8).view(np.uint8).item()
        m8[e_core, e_blk, pos, tile * 128 + (e_di % PB)] = one_fp8
        mT8 = None
        if with_maskT:
            mT8 = m8.reshape(NCORES, NBLK, 128, TPB, 128).transpose(0, 1, 4, 3, 2)
            mT8 = np.ascontiguousarray(mT8).reshape(
                NCORES, NBLK, 128, TPB * 128).view(NP_FP8)

        # idx streams -> [.., 16ch, 128 cols] wrapped layout
        if early_merged:
            ej = e_half * CAP_E + r          # one gather, cols [0, 2*CAP_E/16)
            ecol, lcol0 = ej // 16, 2 * CAP_E // 16
            ech = ej % 16
        else:
            ecol = e_half * (CAP_E // 16) + r // 16
            ech = r % 16
            lcol0 = 2 * (CAP_E // 16)
        lj = lr
        col = np.where(is_e, ecol, lcol0 + e_half * (CAP_L // 16) + lj // 16)
        chan = np.where(is_e, ech, lj % 16)
        v = np.where(is_e, val_e[order], val_l[order]).astype(np.int16)
        idx16 = np.zeros((NCORES, NBLK, 16, 128), np.int16)
        idx16[e_core, e_blk, chan, col] = v
        idx16 = np.broadcast_to(idx16[:, :, None, :, :],
                                (NCORES, NBLK, 8, 16, 128)).reshape(
                                    NCORES, NBLK, 128, 128).copy()
        return idx16, m8.view(NP_FP8), mT8

    for seed in range(64):
        rng = np.random.default_rng(seed)
        perm = rng.permutation(NID)[:N].astype(np.int64)  # orig -> internal
        si = perm[src]
        di = perm[dst]
        gs = grouping(si, di, si % 2, si // 2)                  # SAGE: parity
        cc, kk = si // OWN, si % OWN
        jcls = (kk >= OWN // 2).astype(np.int64)
        half_g = (si >= HALFR).astype(np.int64)
        vEg = cc * (OWN // 2) + (kk % (OWN // 2))   # row in unified tabg_e
        vLg = si - half_g * HALFR
        gg = grouping_el(si, di, half_g, jcls, vEg, vLg, True, True)
        if gs is not None and gg is not None:
            break
    else:
        raise RuntimeError("could not pack edges into halves; increase NBLK")

    # deginv per dst slot
    deg = np.bincount(di, minlength=NID).astype(np.float32)
    dgi = (1.0 / np.maximum(deg, 1.0)).reshape(NCORES, NBLK, PB, 1)

    return perm, gs[0], gs[1], gg[0], gg[1], gg[2], dgi


def kernel(x, src, dst, W_embed, b_embed, Ws1, Wn1, bn1, Ws2, Wn2, bn2,
           Wg, al, ar, bg, W1, b1, W2, b2):
    x = np.asarray(x, np.float32)
    perm, idxS, dcS, idxG, mG, mTG, dgi = _plan(src, dst)

    if "nc" not in _cached:
        _cached["nc"] = _build_bass()
    nc = _cached["nc"]

    # weight preprocessing
    Wg = np.asarray(Wg, np.float32)
    al = np.asarray(al, np.float32)
    ar = np.asarray(ar, np.float32)
    W1 = np.asarray(W1, np.float32)
    WL = np.stack([Wg[:, h * H:(h + 1) * H] @ al[h] for h in range(HEADS)], 1)
    WR = np.stack([Wg[:, h * H:(h + 1) * H] @ ar[h] for h in range(HEADS)], 1)
    b1p = (np.asarray(b1, np.float32) + np.asarray(bg, np.float32) @ W1)
    U = [Wg[:, h * H:(h + 1) * H] @ W1[h * H:(h + 1) * H] for h in range(HEADS)]
    Ulo = np.vstack([U[0], U[1]]).astype(NP_BF16)
    Uhi = np.vstack([U[2], U[3]]).astype(NP_BF16)

    xT = np.zeros((IN, NID), np.float32)
    xT[:, perm] = x.T
    # embed writes table rows p-major (row = p*8+q within each 1024-chunk) so
    # the DMA emits 1KB descriptors; present xT columns in matching order
    xTs = np.ascontiguousarray(
        np.swapaxes(xT.reshape(IN, NCH_ALL // 2, 128, 8), 2, 3).reshape(IN, NID))

    common = {
        "xT": xTs.astype(NP_BF16),
        "wemb": np.asarray(W_embed, np.float32).astype(NP_BF16),
        "bembr": np.asarray(b_embed, np.float32).reshape(1, H),
        "bembc": np.asarray(b_embed, np.float32).reshape(H, 1),
        "ws1": np.asarray(Ws1, np.float32), "wn1": np.asarray(Wn1, np.float32),
        "bn1": np.asarray(bn1, np.float32).reshape(H, 1),
        "ws2": np.asarray(Ws2, np.float32), "wn2": np.asarray(Wn2, np.float32),
        "bn2": np.asarray(bn2, np.float32).reshape(H, 1),
        "wl_in": WL, "wr_in": WR,
        "ulo_in": Ulo, "uhi_in": Uhi,
        "b1p": b1p.reshape(H, 1),
        "w2_in": np.asarray(W2, np.float32),
        "b2c": np.asarray(b2, np.float32).reshape(C, 1),
        "iota_in": np.broadcast_to(np.arange(128, dtype=np.int16), (128, 128)).copy(),
    }
    in_maps = []
    for c in range(NCORES):
        m = dict(common)
        m["xo"] = np.ascontiguousarray(xT[:, c * OWN:(c + 1) * OWN]).astype(NP_BF16)
        m["idx_in"] = np.ascontiguousarray(idxS[c])
        m["dcode_in"] = np.ascontiguousarray(dcS[c])
        m["idx_g_in"] = np.ascontiguousarray(idxG[c])
        m["mask_g_in"] = np.ascontiguousarray(mG[c])
        m["maskT_in"] = np.ascontiguousarray(mTG[c])
        m["dgi_in"] = np.ascontiguousarray(dgi[c])
        in_maps.append(m)

    res = run_bass_kernel_spmd(nc, in_maps, core_ids=list(range(NCORES)))
    full = np.concatenate([res.results[c]["out"] for c in range(NCORES)], 0)
    return full[perm].astype(np.float32)



# revision 77
# speedup vs baseline: 1.0805x; 1.0344x over previous
"""Trainium2 Bass kernel for EnhancedGraphSAGE (embed -> 2x SAGE-mean -> GAT -> MLP).

Self-contained: takes full inputs, shards node-wise across 8 NeuronCores
internally, returns the full [N, C] output.

Design:
- Nodes are relabeled by a random permutation into NID = 8*56*128 internal ids
  (core-major, then 128-dst "blocks"). Each core owns its 56 blocks' dsts.
- Edges grouped by dst block; slots are padded to 128-wide tiles and expanded
  by dma_gather of 256B table rows, then aggregated per dst with TensorE
  matmuls against fp8 one-hot masks (lhsT = mask [128 slots, 128 dsts]).
- SAGE tables pack 2 nodes per 256B row (parity of src selects the 64-col
  window in the matmul), so idx = src//2 always fits int16 and the compact
  AllGather payload needs only a cheap local repack. SAGE masks are generated
  on-chip (DVE is_equal vs an iota) from 2-byte dst codes instead of loading
  14.7MB of one-hot masks per layer.
- AllGather payloads are fp8 (features; GAT also carries 4 bf16 el values in
  the 72B row), and each AG is split in two chunks: chunk 0 (each core's
  first half of rows) fires mid-aggregation of the previous layer so its
  transfer overlaps remaining gather work. GAT additionally keeps a unified
  "early" table of chunk-0 rows so early gather tiles can proceed before
  chunk 1 lands (tiles: 6 early + 5+5 late per block, grouped by src half
  for int16 range).
- GAT: softmax without max-subtraction (exp of leaky_relu bounded; leaky via
  ACT Prelu which shares the exp table set); er[dst] broadcast to edges via
  maskT matmul; z gets a 1e-20 floor via an extra PE accumulate row; per-head
  ex weighting on DVE; Wg folded into W1 on the host (U = Wg_h @ W1_h).
- Embed runs replicated (x in bf16, SWDGE-batched loads), writing the packed
  sage1 table directly.
"""

import numpy as np

import concourse.bacc as bacc
import concourse.bass as bass
import concourse.mybir as mybir
import concourse.tile as tile
from concourse.bass_utils import run_bass_kernel_spmd
from concourse.masks import make_identity

# Problem constants (hardcoded per spec)
N, E, IN, H, HEADS, C = 50000, 800000, 128, 64, 4, 40
SLOPE = 0.2

# Sharding geometry
NCORES = 8
NBLK = 56              # dst blocks per core
PB = 128               # dst slots per block
TPH = 8                # gather tiles per half (1024 idx limit of dma_gather)
TPB = 2 * TPH          # tiles per block
TE = 3                 # early tiles per half-class (chunk-0-only sources)
TL = TPB // 2 - TE     # late tiles per half-class (need the full table)
NE2 = 2 * TE * 128     # early idx per block (both half-classes share a gather)
NL = TL * 128          # late idx per half-class
SLOTH = TPH * 128      # slots per half
S16 = SLOTH // 16      # idx columns in packed [128, S16] layout
OWN = NBLK * PB        # own nodes per core (7168)
NID = NCORES * OWN     # internal id space (57344)
HALFR = NID // 2       # table half split (28672 < 32768)
D = 128                # table row width (bf16 -> 256B rows)
CH = 512               # dense chunk (nodes per matmul)
NCH_OWN = OWN // CH    # 14
NCH_ALL = NID // CH    # 112

F32 = mybir.dt.float32
BF16 = mybir.dt.bfloat16
FP8 = mybir.dt.float8e4
I16 = mybir.dt.int16
NP_BF16 = mybir.dt.np(BF16)
NP_FP8 = mybir.dt.np(FP8)

_cached = {}


def _build_bass(upto=99):
    nc = bacc.Bacc("TRN2", target_bir_lowering=False, debug=False,
                   num_devices=NCORES)

    # ---- I/O ----
    xT = nc.dram_tensor("xT", [IN, NID], BF16, kind="ExternalInput")
    xo = nc.dram_tensor("xo", [IN, OWN], BF16, kind="ExternalInput")
    # SAGE grouping: slots keyed by (dst block, src parity); idx = src//2 into
    # the 2-nodes-per-256B-row packed tables.
    idx_in = nc.dram_tensor("idx_in", [NBLK, 128, 2, S16], I16, kind="ExternalInput")
    dcode_in = nc.dram_tensor("dcode_in", [NBLK, 128, TPB], I16, kind="ExternalInput")
    iota_in = nc.dram_tensor("iota_in", [128, 128], I16, kind="ExternalInput")
    # GAT grouping: (src table half, early/late); 1-node-per-256B-row tables.
    idx_g_in = nc.dram_tensor("idx_g_in", [NBLK, 128, 128], I16, kind="ExternalInput")
    mask_g_in = nc.dram_tensor("mask_g_in", [NBLK, 128, TPB * 128], FP8, kind="ExternalInput")
    maskT_in = nc.dram_tensor("maskT_in", [NBLK, 128, TPB * 128], FP8, kind="ExternalInput")
    dgi_in = nc.dram_tensor("dgi_in", [NBLK, 128, 1], F32, kind="ExternalInput")

    wemb = nc.dram_tensor("wemb", [IN, H], BF16, kind="ExternalInput")
    bembr = nc.dram_tensor("bembr", [1, H], F32, kind="ExternalInput")
    bembc = nc.dram_tensor("bembc", [H, 1], F32, kind="ExternalInput")
    ws1 = nc.dram_tensor("ws1", [H, H], F32, kind="ExternalInput")
    wn1 = nc.dram_tensor("wn1", [H, H], F32, kind="ExternalInput")
    bn1 = nc.dram_tensor("bn1", [H, 1], F32, kind="ExternalInput")
    ws2 = nc.dram_tensor("ws2", [H, H], F32, kind="ExternalInput")
    wn2 = nc.dram_tensor("wn2", [H, H], F32, kind="ExternalInput")
    bn2 = nc.dram_tensor("bn2", [H, 1], F32, kind="ExternalInput")
    wl_in = nc.dram_tensor("wl_in", [H, HEADS], F32, kind="ExternalInput")
    wr_in = nc.dram_tensor("wr_in", [H, HEADS], F32, kind="ExternalInput")
    ulo_in = nc.dram_tensor("ulo_in", [128, H], BF16, kind="ExternalInput")
    uhi_in = nc.dram_tensor("uhi_in", [128, H], BF16, kind="ExternalInput")
    b1p = nc.dram_tensor("b1p", [H, 1], F32, kind="ExternalInput")
    w2_in = nc.dram_tensor("w2_in", [H, C], F32, kind="ExternalInput")
    b2c = nc.dram_tensor("b2c", [C, 1], F32, kind="ExternalInput")

    out = nc.dram_tensor("out", [OWN, C], F32, kind="ExternalOutput")

    with tile.TileContext(nc) as tc:
        with (
            tc.tile_pool(name="wpool", bufs=1) as wp,
            tc.tile_pool(name="sbuf", bufs=3) as sb,
            tc.tile_pool(name="big", bufs=1) as bigp,
            tc.tile_pool(name="psum", bufs=2, space="PSUM") as pp,
            tc.tile_pool(name="dram", bufs=1, space="DRAM") as dram,
        ):
            # ---- constants / weights resident in SBUF ----
            w_emb = wp.tile([IN, H], BF16)
            nc.sync.dma_start(w_emb[:], wemb[:])
            b_embr = wp.tile([1, H], F32)
            nc.sync.dma_start(b_embr[:], bembr[:])
            b_embc = wp.tile([H, 1], F32)
            nc.sync.dma_start(b_embc[:], bembc[:])
            w_s1 = wp.tile([H, H], F32); nc.sync.dma_start(w_s1[:], ws1[:])
            w_n1 = wp.tile([H, H], F32); nc.sync.dma_start(w_n1[:], wn1[:])
            b_n1 = wp.tile([H, 1], F32); nc.sync.dma_start(b_n1[:], bn1[:])
            w_s2 = wp.tile([H, H], F32); nc.sync.dma_start(w_s2[:], ws2[:])
            w_n2 = wp.tile([H, H], F32); nc.sync.dma_start(w_n2[:], wn2[:])
            b_n2 = wp.tile([H, 1], F32); nc.sync.dma_start(b_n2[:], bn2[:])
            w_lr = wp.tile([H, 2 * HEADS], F32)
            nc.sync.dma_start(w_lr[:, 0:HEADS], wl_in[:])
            nc.sync.dma_start(w_lr[:, HEADS:], wr_in[:])
            u_lo = wp.tile([128, H], BF16); nc.sync.dma_start(u_lo[:], ulo_in[:])
            u_hi = wp.tile([128, H], BF16); nc.sync.dma_start(u_hi[:], uhi_in[:])
            b_1p = wp.tile([H, 1], F32); nc.sync.dma_start(b_1p[:], b1p[:])
            w_2 = wp.tile([H, C], F32); nc.sync.dma_start(w_2[:], w2_in[:])
            b_2 = wp.tile([C, 1], F32); nc.sync.dma_start(b_2[:], b2c[:])

            ones1 = wp.tile([1, 128], F32)
            nc.vector.memset(ones1[:], 1.0)
            iota_sb = wp.tile([128, 128], I16)
            nc.sync.dma_start(iota_sb[:], iota_in[:])
            epsz = wp.tile([1, HEADS * H + HEADS], F32)
            nc.vector.memset(epsz[:, 0:HEADS * H], 0.0)
            nc.vector.memset(epsz[:, HEADS * H:], 1e-20)
            id64f = wp.tile([64, 64], F32)
            make_identity(nc, id64f[:])
            id128f = wp.tile([128, 128], F32)
            make_identity(nc, id128f[:])
            id128b = wp.tile([128, 128], BF16)
            nc.vector.tensor_copy(id128b[:], id128f[:])
            id40f = wp.tile([40, 40], F32)
            make_identity(nc, id40f[:])

            # deginv: per-partition scalar per block -> SBUF [128, NBLK]
            dgi_sb = bigp.tile([128, NBLK], F32)
            nc.sync.dma_start(dgi_sb[:], dgi_in[:].rearrange("b p one -> p (b one)"))

            # persistent feature planes
            h1T = bigp.tile([H, OWN], F32, tag="hT", bufs=2)  # feat-major planes
            h2T = bigp.tile([H, OWN], F32, tag="hT", bufs=2)
            h3T = bigp.tile([H, OWN], F32, tag="hT", bufs=2)
            neighT = bigp.tile([H, OWN], F32)
            er_all = bigp.tile([128, NBLK, HEADS], BF16)
            og_nm = bigp.tile([128, NBLK, 2 * H * 2], BF16)  # node-major GAT out

            # DRAM tables. SAGE tables pack 2 nodes per 256B row, so the
            # compact [*, 64] AllGather output IS the gather table. The GAT
            # table needs 68 cols/node -> 256B rows + a repack after the AG.
            tab1 = dram.tile([HALFR, D], BF16)
            mine2 = dram.tile([OWN, 64], FP8)
            ag1a = dram.tile([NID // 2, 64], FP8, addr_space="Shared")
            ag1b = dram.tile([NID // 2, 64], FP8, addr_space="Shared")
            tab2q = dram.tile([NCORES, OWN // 2, 256], FP8)
            mineg = dram.tile([OWN, 72], FP8)
            agga = dram.tile([NID // 2, 72], FP8, addr_space="Shared")
            aggb = dram.tile([NID // 2, 72], FP8, addr_space="Shared")
            tabg_e = dram.tile([NID // 2, 256], FP8)
            tabga = dram.tile([4, OWN, 256], FP8)
            tabgb = dram.tile([4, OWN, 256], FP8)
            tab1v = tab1[:]

            # ================= P1: embed =================
            # full table (replicated): tab1 row r = bf16(h1 of nodes 2r, 2r+1)
            for ch2 in range(NCH_ALL // 2):
                xb = sb.tile([IN, 2 * CH], BF16, tag="xb")
                nc.gpsimd.dma_start(xb[:], xT[:, ch2 * 2 * CH:(ch2 + 1) * 2 * CH])
                stg = sb.tile([128, 8, H], BF16, tag="stg1")
                for sub in range(2):
                    pe = pp.tile([128, 4, H], F32, space="PSUM", tag="psA", bufs=4)
                    for q in range(4):
                        nc.tensor.matmul(
                            pe[:, q, :],
                            xb[:, sub * CH + q * 128:sub * CH + (q + 1) * 128],
                            w_emb[:], start=True, stop=False)
                        nc.tensor.matmul(pe[:, q, :], ones1[0:1, 0:128],
                                         b_embr[0:1, :], start=False, stop=True)
                    nc.vector.tensor_copy(stg[:, sub * 4:(sub + 1) * 4, :], pe[:])
                nc.scalar.dma_start(
                    tab1[ch2 * CH:(ch2 + 1) * CH, :].rearrange(
                        "r (two d) -> (r two) d", two=2).rearrange(
                        "(p q) d -> p q d", q=8), stg[:])
            # own features, feat-major (f32)
            for ch in range(NCH_OWN):
                xb2 = sb.tile([IN, CH], BF16, tag="xb")
                nc.gpsimd.dma_start(xb2[:], xo[:, ch * CH:(ch + 1) * CH])
                ph = pp.tile([H, CH], F32, space="PSUM", tag="psB", bufs=4)
                nc.tensor.matmul(ph[:], w_emb[:], xb2[:], start=True, stop=True)
                nc.scalar.activation(h1T[:, ch * CH:(ch + 1) * CH], ph[:],
                                     mybir.ActivationFunctionType.Identity,
                                     bias=b_embc[:], scale=1.0)

            # ============== SAGE layer helper ==============
            def sage_agg(table, chunk_cb=None):
                """Aggregate neighbor means into neighT (feat-major, f32).

                chunk_cb(ch) runs after each 4-block group's neighT is ready so
                the dense layer + row writes overlap the remaining gathers.
                Table is parity-packed: tiles 0:TPH hold even-src slots (cols
                0:H of the gathered rows), tiles TPH:TPB odd-src (cols H:2H).
                """
                gdt = table.dtype
                delem = 256 if gdt == FP8 else D
                it4 = None
                for b in range(NBLK):
                    if b % 4 == 0:
                        it4 = sb.tile([128, 4, 2, S16], I16, tag="it", bufs=2)
                        nc.sync.dma_start(it4[:], idx_in[b:b + 4].rearrange(
                            "q p h s -> p q h s"))
                    it = it4
                    dc = sb.tile([128, TPB], I16, tag="dc", bufs=4)
                    nc.sync.dma_start(dc[:], dcode_in[b])
                    mk = sb.tile([128, TPB * 128], FP8, tag="mk", bufs=4)
                    nc.vector.tensor_tensor(
                        mk[:].rearrange("p (t d) -> p t d", t=TPB),
                        dc[:].rearrange("p (t o) -> p t o", o=1).to_broadcast(
                            [128, TPB, 128]),
                        iota_sb[:].rearrange("p (o d) -> p o d", o=1).to_broadcast(
                            [128, TPB, 128]),
                        mybir.AluOpType.is_equal)
                    g = sb.tile([128, TPB, delem], gdt, tag="g", bufs=5)
                    nc.gpsimd.dma_gather(g[:, 0:TPH, :], table,
                                         it[:, b % 4, 0, :], SLOTH, SLOTH, delem)
                    nc.gpsimd.dma_gather(g[:, TPH:TPB, :], table,
                                         it[:, b % 4, 1, :], SLOTH, SLOTH, delem)
                    pa = pp.tile([128, H], F32, space="PSUM", tag="psA", bufs=4)
                    for t in range(TPB):
                        off = 0 if t < TPH else H
                        nc.tensor.matmul(pa[:], mk[:, t * 128:(t + 1) * 128],
                                         g[:, t, off:off + H],
                                         start=(t == 0), stop=(t == TPB - 1))
                    nb = sb.tile([128, H], F32, tag="nb")
                    nc.vector.tensor_scalar_mul(nb[:], pa[:], dgi_sb[:, b:b + 1])
                    pt = pp.tile([H, 128], F32, space="PSUM", tag="psA", bufs=4)
                    nc.tensor.transpose(pt[:], nb[:], id128f[:])
                    nc.scalar.activation(neighT[:, b * 128:(b + 1) * 128], pt[:],
                                         mybir.ActivationFunctionType.Identity)
                    if chunk_cb is not None and b % 4 == 3:
                        chunk_cb(b // 4)

            def write_row_blk(hT_src, stg4, q, b, with_el):
                """Transpose one block's feat-major features into rows."""
                ptr = pp.tile([128, H], F32, space="PSUM", tag="psA", bufs=4)
                nc.tensor.transpose(ptr[:], hT_src[:, b * 128:(b + 1) * 128],
                                    id64f[:])
                nc.vector.tensor_copy(stg4[:, q, 0:H], ptr[:])
                if with_el:
                    pel = pp.tile([128, 2 * HEADS], F32, space="PSUM", tag="psA",
                                  bufs=4)
                    nc.tensor.matmul(pel[:], hT_src[:, b * 128:(b + 1) * 128],
                                     w_lr[:], start=True, stop=True)
                    nc.vector.tensor_copy(
                        stg4[:, q, H:H + 2 * HEADS].bitcast(BF16),
                        pel[:, 0:HEADS])
                    nc.vector.tensor_copy(er_all[:, b, :], pel[:, HEADS:])

            def ag_chunk(mine, agc, j):
                """AllGather chunk j: every rank's rows [j*OWN/2,(j+1)*OWN/2)
                concatenate into the contiguous chunk buffer."""
                nc.gpsimd.collective_compute(
                    "AllGather", mybir.AluOpType.bypass,
                    replica_groups=[list(range(NCORES))],
                    ins=[mine[j * OWN // 2:(j + 1) * OWN // 2, :].opt()],
                    outs=[agc[:].opt()],
                )

            def sage_fused_cb(hT_in, w_s, w_n, b_n, hT_out, mine, width,
                              with_el=False, on_c0=None):
                def cb(ch):
                    pd = pp.tile([H, CH], F32, space="PSUM", tag="psB", bufs=4)
                    nc.tensor.matmul(pd[:], w_s[:], hT_in[:, ch * CH:(ch + 1) * CH],
                                     start=True, stop=False)
                    nc.tensor.matmul(pd[:], w_n[:], neighT[:, ch * CH:(ch + 1) * CH],
                                     start=False, stop=True)
                    nc.scalar.activation(hT_out[:, ch * CH:(ch + 1) * CH], pd[:],
                                         mybir.ActivationFunctionType.Relu,
                                         bias=b_n[:], scale=1.0)
                    stg4 = sb.tile([128, 4, width], mine.dtype, tag="stg2",
                                   bufs=2)
                    for q in range(4):
                        write_row_blk(hT_out, stg4, q, ch * 4 + q, with_el)
                    nc.sync.dma_start(
                        mine[ch * CH:(ch + 1) * CH, :].rearrange(
                            "(q p) w -> p q w", p=128), stg4[:])
                    if on_c0 is not None and ch == NCH_OWN // 2 + 1:
                        on_c0()
                return cb

            def sage_agg_el(tabE, tabL, idx2, mask2, chunk_cb):
                """Early/late variant: tiles 0:TE p0-early, TE:2TE p1-early
                (sources living in AG chunk 0, read from tabE), then TL-tile
                late groups per parity from the full table tabL."""
                for b in range(NBLK):
                    it = sb.tile([128, 128], I16, tag="it2", bufs=3)
                    nc.sync.dma_start(it[:], idx2[b])
                    mk = sb.tile([128, TPB * 128], FP8, tag="mk", bufs=4)
                    nc.sync.dma_start(mk[:], mask2[b])
                    g = sb.tile([128, TPB, D], BF16, tag="g", bufs=5)
                    nc.gpsimd.dma_gather(g[:, 0:2 * TE, :], tabE,
                                         it[:, 0:NE2 // 16], NE2, NE2, D)
                    nc.gpsimd.dma_gather(g[:, 2 * TE:2 * TE + TL, :], tabL,
                                         it[:, NE2 // 16:(NE2 + NL) // 16], NL, NL, D)
                    nc.gpsimd.dma_gather(g[:, 2 * TE + TL:TPB, :], tabL,
                                         it[:, (NE2 + NL) // 16:(NE2 + 2 * NL) // 16],
                                         NL, NL, D)
                    pa = pp.tile([128, H], F32, space="PSUM", tag="psA", bufs=4)
                    for t in range(TPB):
                        off = 0 if (t < TE or 2 * TE <= t < 2 * TE + TL) else H
                        nc.tensor.matmul(pa[:], mk[:, t * 128:(t + 1) * 128],
                                         g[:, t, off:off + H],
                                         start=(t == 0), stop=(t == TPB - 1))
                    nb = sb.tile([128, H], F32, tag="nb")
                    nc.vector.tensor_scalar_mul(nb[:], pa[:], dgi_sb[:, b:b + 1])
                    pt = pp.tile([H, 128], F32, space="PSUM", tag="psA", bufs=4)
                    nc.tensor.transpose(pt[:], nb[:], id128f[:])
                    nc.scalar.activation(neighT[:, b * 128:(b + 1) * 128], pt[:],
                                         mybir.ActivationFunctionType.Identity)
                    if chunk_cb is not None and b % 4 == 3:
                        chunk_cb(b // 4)

            # ================= SAGE 1 =================
            def sage1_c0():
                ag_chunk(mine2, ag1a, 0)
                nc.sync.dma_start(
                    tab2q[:, 0:OWN // 4, 0:128],
                    ag1a[:].rearrange("(c r two) d -> c r (two d)",
                                      c=NCORES, two=2))
            if upto >= 2:
                sage_agg(tab1v,
                         sage_fused_cb(h1T, w_s1, w_n1, b_n1, h2T, mine2, 64,
                                       on_c0=sage1_c0)
                         if upto >= 3 else None)
            if upto >= 3:
                ag_chunk(mine2, ag1b, 1)
                nc.sync.dma_start(
                    tab2q[:, OWN // 4:OWN // 2, 0:128],
                    ag1b[:].rearrange("(c r two) d -> c r (two d)",
                                      c=NCORES, two=2))

            # ================= SAGE 2 =================
            def sage2_c0():
                ag_chunk(mineg, agga, 0)
                # early GAT table: all chunk-0 rows (fits int16 indexing)
                nc.scalar.dma_start(tabg_e[:, 0:72], agga[:])
            if upto >= 4:
                sage_agg(tab2q[:].rearrange("c r d -> (c r) d"),
                         sage_fused_cb(h2T, w_s2, w_n2, b_n2, h3T, mineg, 72,
                                       with_el=True, on_c0=sage2_c0))
                ag_chunk(mineg, aggb, 1)
                # unified GAT tables from both chunks
                nc.sync.dma_start(
                    tabga[:, 0:OWN // 2, 0:72],
                    agga[0:HALFR // 2, :].rearrange("(c r) d -> c r d", c=4))
                nc.scalar.dma_start(
                    tabgb[:, 0:OWN // 2, 0:72],
                    agga[HALFR // 2:NID // 2, :].rearrange("(c r) d -> c r d", c=4))
                nc.sync.dma_start(
                    tabga[:, OWN // 2:OWN, 0:72],
                    aggb[0:HALFR // 2, :].rearrange("(c r) d -> c r d", c=4))
                nc.scalar.dma_start(
                    tabgb[:, OWN // 2:OWN, 0:72],
                    aggb[HALFR // 2:NID // 2, :].rearrange("(c r) d -> c r d", c=4))

            # ------- GAT dense + classifier (per 4-block chunk) -------
            def og_stage(ch, half):
                stgT = sb.tile([128, CH], BF16, tag=f"ogs{half}", bufs=2)
                for q in range(4):
                    b = ch * 4 + q
                    ptg = pp.tile([128, 128], BF16, space="PSUM", tag="psA", bufs=4)
                    nc.tensor.transpose(
                        ptg[:], og_nm[:, b, half * 128:(half + 1) * 128], id128b[:])
                    nc.vector.tensor_copy(stgT[:, q * 128:(q + 1) * 128], ptg[:])
                return stgT

            def gat_tail(ch):
                og_loS = og_stage(ch, 0)
                og_hiS = og_stage(ch, 1)
                p4 = pp.tile([H, CH], F32, space="PSUM", tag="psB", bufs=4)
                nc.tensor.matmul(p4[:], u_lo[:], og_loS[:],
                                 start=True, stop=False)
                nc.tensor.matmul(p4[:], u_hi[:], og_hiS[:],
                                 start=False, stop=True)
                h4 = sb.tile([H, CH], F32, tag="h4")
                nc.scalar.activation(h4[:], p4[:],
                                     mybir.ActivationFunctionType.Relu,
                                     bias=b_1p[:], scale=1.0)
                plg = pp.tile([C, CH], F32, space="PSUM", tag="psB", bufs=4)
                nc.tensor.matmul(plg[:], w_2[:], h4[:], start=True, stop=True)
                lg = sb.tile([C, CH], F32, tag="lg")
                nc.scalar.activation(lg[:], plg[:],
                                     mybir.ActivationFunctionType.Identity,
                                     bias=b_2[:], scale=1.0)
                ostg = sb.tile([128, 4, C], F32, tag="ostg")
                for q in range(4):
                    plt = pp.tile([128, C], F32, space="PSUM", tag="psA", bufs=4)
                    nc.tensor.transpose(plt[:], lg[:, q * 128:(q + 1) * 128], id40f[:])
                    nc.scalar.activation(ostg[:, q, :], plt[:],
                                         mybir.ActivationFunctionType.Identity)
                nc.sync.dma_start(
                    out[ch * CH:(ch + 1) * CH, :].rearrange("(q p) c -> p q c", p=128),
                    ostg[:])

            # ================= GAT aggregation =================
            _noW = _noER = _noPG = _noTR = False
            tabgaf = tabga[:].rearrange("c r d -> (c r) d")
            tabgbf = tabgb[:].rearrange("c r d -> (c r) d")
            for b in range(NBLK if upto >= 5 else 0):
                it4g = sb.tile([128, 128], I16, tag="it2", bufs=3)
                nc.sync.dma_start(it4g[:], idx_g_in[b])
                mk = sb.tile([128, TPB * 128], FP8, tag="mk", bufs=4)
                nc.sync.dma_start(mk[:], mask_g_in[b])
                mt = sb.tile([128, TPB * 128], FP8, tag="mt", bufs=3)
                nc.sync.dma_start(mt[:], maskT_in[b])
                g = sb.tile([128, TPB, 256], FP8, tag="g", bufs=5)
                nc.gpsimd.dma_gather(g[:, 0:2 * TE, :], tabg_e[:],
                                     it4g[:, 0:NE2 // 16], NE2, NE2, 256)
                nc.gpsimd.dma_gather(g[:, 2 * TE:2 * TE + TL, :], tabgaf,
                                     it4g[:, NE2 // 16:(NE2 + NL) // 16],
                                     NL, NL, 256)
                nc.gpsimd.dma_gather(g[:, 2 * TE + TL:TPB, :], tabgbf,
                                     it4g[:, (NE2 + NL) // 16:128],
                                     NL, NL, 256)
                # er broadcast to edge slots via maskT matmuls
                perb = pp.tile([128, TPB, HEADS], F32, space="PSUM", tag="psA", bufs=4)
                for t in range(TPB):
                    nc.tensor.matmul(perb[:, t, :], mt[:, t * 128:(t + 1) * 128],
                                     er_all[:, b, :], start=True, stop=True)
                # e = leaky_relu(el + er); ex = exp(e)  (no Lrelu table: max(x, .2x))
                ee = sb.tile([128, TPB, HEADS], BF16, tag="ee")
                nc.vector.tensor_add(ee[:], g[:, :, H:H + 2 * HEADS].bitcast(BF16), perb[:])
                eeL = sb.tile([128, TPB, HEADS], BF16, tag="et")
                nc.scalar.activation(eeL[:], ee[:],
                                     mybir.ActivationFunctionType.Prelu,
                                     alpha=SLOPE)
                wst = sb.tile([128, TPB, HEADS * H + HEADS], BF16, tag="wst", bufs=3)
                nc.scalar.activation(wst[:, :, HEADS * H:], eeL[:],
                                     mybir.ActivationFunctionType.Exp)
                pg = pp.tile([128, HEADS * H + HEADS], F32, space="PSUM", tag="psB", bufs=4)
                HT = TPB // 2
                for half in range(2):
                    tsl = slice(half * HT, (half + 1) * HT)
                    if _noW:
                        if b == 0 and half == 0:
                            nc.vector.memset(wst[:, :, 0:HEADS * H], 0.5)
                    else:
                        # fused per-head weighting: wst[p,t,h,f] = g[p,t,f]*ex[p,t,h]
                        nc.vector.tensor_mul(
                            wst[:, tsl, 0:HEADS * H].rearrange(
                                "p t (h f) -> p t h f", h=HEADS),
                            g[:, tsl, 0:H].rearrange(
                                "p t (o f) -> p t o f", o=1).to_broadcast(
                                    [128, HT, HEADS, H]),
                            wst[:, tsl, HEADS * H:].rearrange(
                                "p t (h o) -> p t h o", o=1).to_broadcast(
                                    [128, HT, HEADS, H]))
                    for t in range(half * HT, (half + 1) * HT):
                        nc.tensor.matmul(pg[:], mk[:, t * 128:(t + 1) * 128],
                                         wst[:, t, :], start=(t == 0),
                                         stop=False)
                nc.tensor.matmul(pg[:], ones1[0:1, 0:128],
                                 epsz[0:1, :], start=False, stop=True)
                # normalize by z and transpose for the dense phase
                zi = sb.tile([128, HEADS], F32, tag="zi")
                nc.vector.reciprocal(zi[:], pg[:, HEADS * H:])
                nc.vector.tensor_mul(
                    og_nm[:, b, :].rearrange("p (h f) -> p h f", h=HEADS),
                    pg[:, 0:HEADS * H].rearrange("p (h f) -> p h f", h=HEADS),
                    zi[:].to_broadcast([128, HEADS, H]))
            for ch in range(NCH_OWN if upto >= 6 else 0):
                gat_tail(ch)

            if upto < 6:
                zo = sb.tile([128, NBLK, C], F32, tag="zo")
                nc.vector.memset(zo[:], 0.0)
                nc.sync.dma_start(
                    out[:].rearrange("(q p) c -> p q c", p=128), zo[:])

    nc.compile()
    return nc


def _plan(src, dst):
    """Host-side graph partitioning. Returns per-core index/mask arrays."""
    src = np.asarray(src).astype(np.int64)
    dst = np.asarray(dst).astype(np.int64)

    def grouping(si, di, half, val):
        """Slot layout for one (half-assignment, idx-value) scheme."""
        gblk = di // PB
        grp = gblk * 2 + half
        cnt = np.bincount(grp, minlength=NCORES * NBLK * 2)
        if cnt.max() > SLOTH:
            return None
        order = np.lexsort((si, grp))
        g_sorted = grp[order]
        starts = np.zeros(NCORES * NBLK * 2 + 1, np.int64)
        np.cumsum(cnt, out=starts[1:])
        j_in_grp = np.arange(E, dtype=np.int64) - starts[g_sorted]
        e_di = di[order]
        e_half = half[order]
        e_gblk = gblk[order]
        e_core = e_gblk // NBLK
        e_blk = e_gblk % NBLK

        idx16 = np.zeros((NCORES, NBLK, 16, 2, S16), np.int16)
        idx16[e_core, e_blk, j_in_grp % 16, e_half, j_in_grp // 16] = \
            val[order].astype(np.int16)
        idx16 = np.broadcast_to(idx16[:, :, None, :, :, :],
                                (NCORES, NBLK, 8, 16, 2, S16)).reshape(
                                    NCORES, NBLK, 128, 2, S16).copy()

        t_of = (e_half * TPH + j_in_grp // 128).astype(np.int64)
        p_of = (j_in_grp % 128).astype(np.int64)
        d_of = (e_di % PB).astype(np.int64)
        # dst codes: one dst column (or 255 = empty slot) per (partition, tile)
        dcode = np.full((NCORES, NBLK, 128, TPB), 255, np.int16)
        dcode[e_core, e_blk, p_of, t_of] = d_of.astype(np.int16)
        return idx16, dcode

    def grouping_el(si, di, half, jcls, val_e, val_l, early_merged, with_maskT):
        """Early/late slot layout. Early tiles hold only chunk-0 (jcls==0)
        sources; overflow and all chunk-1 sources go to the late tiles."""
        CAP_E, CAP_L = TE * 128, TL * 128
        NG = NCORES * NBLK * 2
        gblk = di // PB
        key = gblk * 2 + half
        order = np.lexsort((si, jcls, key))
        k_s = key[order]
        cnt = np.bincount(key, minlength=NG)
        starts = np.zeros(NG + 1, np.int64)
        np.cumsum(cnt, out=starts[1:])
        r = np.arange(E, dtype=np.int64) - starts[k_s]
        j_s = jcls[order]
        j0cnt = np.bincount(key[jcls == 0], minlength=NG)
        etk = np.minimum(j0cnt, CAP_E)
        is_e = (j_s == 0) & (r < CAP_E)
        lr = r - etk[k_s]
        if int(np.where(is_e, 0, lr).max()) >= CAP_L:
            return None
        e_half = half[order]
        e_di = di[order]
        e_gblk = gblk[order]
        e_core = e_gblk // NBLK
        e_blk = e_gblk % NBLK

        tile = np.where(is_e, e_half * TE + r // 128,
                        2 * TE + e_half * TL + lr // 128)
        pos = np.where(is_e, r % 128, lr % 128)
        m8 = np.zeros((NCORES, NBLK, 128, TPB * 128), np.uint8)
        one_fp8 = np.array(1.0, NP_FP8).view(np.uint8).item()
        m8[e_core, e_blk, pos, tile * 128 + (e_di % PB)] = one_fp8
        mT8 = None
        if with_maskT:
            mT8 = m8.reshape(NCORES, NBLK, 128, TPB, 128).transpose(0, 1, 4, 3, 2)
            mT8 = np.ascontiguousarray(mT8).reshape(
                NCORES, NBLK, 128, TPB * 128).view(NP_FP8)

        # idx streams -> [.., 16ch, 128 cols] wrapped layout
        if early_merged:
            ej = e_half * CAP_E + r          # one gather, cols [0, 2*CAP_E/16)
            ecol, lcol0 = ej // 16, 2 * CAP_E // 16
            ech = ej % 16
        else:
            ecol = e_half * (CAP_E // 16) + r // 16
            ech = r % 16
            lcol0 = 2 * (CAP_E // 16)
        lj = lr
        col = np.where(is_e, ecol, lcol0 + e_half * (CAP_L // 16) + lj // 16)
        chan = np.where(is_e, ech, lj % 16)
        v = np.where(is_e, val_e[order], val_l[order]).astype(np.int16)
        idx16 = np.zeros((NCORES, NBLK, 16, 128), np.int16)
        idx16[e_core, e_blk, chan, col] = v
        idx16 = np.broadcast_to(idx16[:, :, None, :, :],
                                (NCORES, NBLK, 8, 16, 128)).reshape(
                                    NCORES, NBLK, 128, 128).copy()
        return idx16, m8.view(NP_FP8), mT8

    for seed in range(64):
        rng = np.random.default_rng(seed)
        perm = rng.permutation(NID)[:N].astype(np.int64)  # orig -> internal
        si = perm[src]
        di = perm[dst]
        gs = grouping(si, di, si % 2, si // 2)                  # SAGE: parity
        cc, kk = si // OWN, si % OWN
        jcls = (kk >= OWN // 2).astype(np.int64)
        half_g = (si >= HALFR).astype(np.int64)
        vEg = cc * (OWN // 2) + (kk % (OWN // 2))   # row in unified tabg_e
        vLg = si - half_g * HALFR
        gg = grouping_el(si, di, half_g, jcls, vEg, vLg, True, True)
        if gs is not None and gg is not None:
            break
    else:
        raise RuntimeError("could not pack edges into halves; increase NBLK")

    # deginv per dst slot
    deg = np.bincount(di, minlength=NID).astype(np.float32)
    dgi = (1.0 / np.maximum(deg, 1.0)).reshape(NCORES, NBLK, PB, 1)

    return perm, gs[0], gs[1], gg[0], gg[1], gg[2], dgi


def kernel(x, src, dst, W_embed, b_embed, Ws1, Wn1, bn1, Ws2, Wn2, bn2,
           Wg, al, ar, bg, W1, b1, W2, b2):
    x = np.asarray(x, np.float32)
    perm, idxS, dcS, idxG, mG, mTG, dgi = _plan(src, dst)

    if "nc" not in _cached:
        _cached["nc"] = _build_bass()
    nc = _cached["nc"]

    # weight preprocessing
    Wg = np.asarray(Wg, np.float32)
    al = np.asarray(al, np.float32)
    ar = np.asarray(ar, np.float32)
    W1 = np.asarray(W1, np.float32)
    WL = np.stack([Wg[:, h * H:(h + 1) * H] @ al[h] for h in range(HEADS)], 1)
    WR = np.stack([Wg[:, h * H:(h + 1) * H] @ ar[h] for h in range(HEADS)], 1)
    b1p = (np.asarray(b1, np.float32) + np.asarray(bg, np.float32) @ W1)
    U = [Wg[:, h * H:(h + 1) * H] @ W1[h * H:(h + 1) * H] for h in range(HEADS)]
    Ulo = np.vstack([U[0], U[1]]).astype(NP_BF16)
    Uhi = np.vstack([U[2], U[3]]).astype(NP_BF16)

    xT = np.zeros((IN, NID), np.float32)
    xT[:, perm] = x.T
    # embed writes table rows p-major (row = p*8+q within each 1024-chunk) so
    # the DMA emits 1KB descriptors; present xT columns in matching order
    xTs = np.ascontiguousarray(
        np.swapaxes(xT.reshape(IN, NCH_ALL // 2, 128, 8), 2, 3).reshape(IN, NID))

    common = {
        "xT": xTs.astype(NP_BF16),
        "wemb": np.asarray(W_embed, np.float32).astype(NP_BF16),
        "bembr": np.asarray(b_embed, np.float32).reshape(1, H),
        "bembc": np.asarray(b_embed, np.float32).reshape(H, 1),
        "ws1": np.asarray(Ws1, np.float32), "wn1": np.asarray(Wn1, np.float32),
        "bn1": np.asarray(bn1, np.float32).reshape(H, 1),
        "ws2": np.asarray(Ws2, np.float32), "wn2": np.asarray(Wn2, np.float32),
        "bn2": np.asarray(bn2, np.float32).reshape(H, 1),
        "wl_in": WL, "wr_in": WR,
        "ulo_in": Ulo, "uhi_in": Uhi,
        "b1p": b1p.reshape(H, 1),
        "w2_in": np.asarray(W2, np.float32),
        "b2c": np.asarray(b2, np.float32).reshape(C, 1),
        "iota_in": np.broadcast_to(np.arange(128, dtype=np.int16), (128, 128)).copy(),
    }
    in_maps = []
    for c in range(NCORES):
        m = dict(common)
        m["xo"] = np.ascontiguousarray(xT[:, c * OWN:(c + 1) * OWN]).astype(NP_BF16)
        m["idx_in"] = np.ascontiguousarray(idxS[c])
        m["dcode_in"] = np.ascontiguousarray(dcS[c])
        m["idx_g_in"] = np.ascontiguousarray(idxG[c])
        m["mask_g_in"] = np.ascontiguousarray(mG[c])
        m["maskT_in"] = np.ascontiguousarray(mTG[c])
        m["dgi_in"] = np.ascontiguousarray(dgi[c])
        in_maps.append(m)

    res = run_bass_kernel_spmd(nc, in_maps, core_ids=list(range(NCORES)))
    full = np.concatenate([res.results[c]["out"] for c in range(NCORES)], 0)
    return full[perm].astype(np.float32)



# revision 78
# speedup vs baseline: 1.0817x; 1.0011x over previous
"""Trainium2 Bass kernel for EnhancedGraphSAGE (embed -> 2x SAGE-mean -> GAT -> MLP).

Self-contained: takes full inputs, shards node-wise across 8 NeuronCores
internally, returns the full [N, C] output.

Design:
- Nodes are relabeled by a random permutation into NID = 8*56*128 internal ids
  (core-major, then 128-dst "blocks"). Each core owns its 56 blocks' dsts.
- Edges grouped by dst block; slots are padded to 128-wide tiles and expanded
  by dma_gather of 256B table rows, then aggregated per dst with TensorE
  matmuls against fp8 one-hot masks (lhsT = mask [128 slots, 128 dsts]).
- SAGE tables pack 2 nodes per 256B row (parity of src selects the 64-col
  window in the matmul), so idx = src//2 always fits int16 and the compact
  AllGather payload needs only a cheap local repack. SAGE masks are generated
  on-chip (DVE is_equal vs an iota) from 2-byte dst codes instead of loading
  14.7MB of one-hot masks per layer.
- AllGather payloads are fp8 (features; GAT also carries 4 bf16 el values in
  the 72B row), and each AG is split in two chunks: chunk 0 (each core's
  first half of rows) fires mid-aggregation of the previous layer so its
  transfer overlaps remaining gather work. GAT additionally keeps a unified
  "early" table of chunk-0 rows so early gather tiles can proceed before
  chunk 1 lands (tiles: 6 early + 5+5 late per block, grouped by src half
  for int16 range).
- GAT: softmax without max-subtraction (exp of leaky_relu bounded; leaky via
  ACT Prelu which shares the exp table set); er[dst] broadcast to edges via
  maskT matmul; z gets a 1e-20 floor via an extra PE accumulate row; per-head
  ex weighting on DVE; Wg folded into W1 on the host (U = Wg_h @ W1_h).
- Embed runs replicated (x in bf16, SWDGE-batched loads), writing the packed
  sage1 table directly.
"""

import numpy as np

import concourse.bacc as bacc
import concourse.bass as bass
import concourse.mybir as mybir
import concourse.tile as tile
from concourse.bass_utils import run_bass_kernel_spmd
from concourse.masks import make_identity

# Problem constants (hardcoded per spec)
N, E, IN, H, HEADS, C = 50000, 800000, 128, 64, 4, 40
SLOPE = 0.2

# Sharding geometry
NCORES = 8
NBLK = 56              # dst blocks per core
PB = 128               # dst slots per block
TPH = 8                # gather tiles per half (1024 idx limit of dma_gather)
TPB = 2 * TPH          # tiles per block
TE = 3                 # early tiles per half-class (chunk-0-only sources)
TL = TPB // 2 - TE     # late tiles per half-class (need the full table)
NE2 = 2 * TE * 128     # early idx per block (both half-classes share a gather)
NL = TL * 128          # late idx per half-class
SLOTH = TPH * 128      # slots per half
S16 = SLOTH // 16      # idx columns in packed [128, S16] layout
OWN = NBLK * PB        # own nodes per core (7168)
NID = NCORES * OWN     # internal id space (57344)
HALFR = NID // 2       # table half split (28672 < 32768)
D = 128                # table row width (bf16 -> 256B rows)
CH = 512               # dense chunk (nodes per matmul)
NCH_OWN = OWN // CH    # 14
NCH_ALL = NID // CH    # 112

F32 = mybir.dt.float32
BF16 = mybir.dt.bfloat16
FP8 = mybir.dt.float8e4
I16 = mybir.dt.int16
NP_BF16 = mybir.dt.np(BF16)
NP_FP8 = mybir.dt.np(FP8)

_cached = {}


def _build_bass(upto=99):
    nc = bacc.Bacc("TRN2", target_bir_lowering=False, debug=False,
                   num_devices=NCORES)

    # ---- I/O ----
    xT = nc.dram_tensor("xT", [IN, NID], BF16, kind="ExternalInput")
    xo = nc.dram_tensor("xo", [IN, OWN], BF16, kind="ExternalInput")
    # SAGE grouping: slots keyed by (dst block, src parity); idx = src//2 into
    # the 2-nodes-per-256B-row packed tables.
    idx_in = nc.dram_tensor("idx_in", [NBLK, 128, 2, S16], I16, kind="ExternalInput")
    dcode_in = nc.dram_tensor("dcode_in", [NBLK, 128, TPB], I16, kind="ExternalInput")
    iota_in = nc.dram_tensor("iota_in", [128, 128], I16, kind="ExternalInput")
    # GAT grouping: (src table half, early/late); 1-node-per-256B-row tables.
    idx_g_in = nc.dram_tensor("idx_g_in", [NBLK, 128, 128], I16, kind="ExternalInput")
    mask_g_in = nc.dram_tensor("mask_g_in", [NBLK, 128, TPB * 128], FP8, kind="ExternalInput")
    maskT_in = nc.dram_tensor("maskT_in", [NBLK, 128, TPB * 128], FP8, kind="ExternalInput")
    dgi_in = nc.dram_tensor("dgi_in", [NBLK, 128, 1], F32, kind="ExternalInput")

    wemb = nc.dram_tensor("wemb", [IN, H], BF16, kind="ExternalInput")
    bembr = nc.dram_tensor("bembr", [1, H], F32, kind="ExternalInput")
    bembc = nc.dram_tensor("bembc", [H, 1], F32, kind="ExternalInput")
    ws1 = nc.dram_tensor("ws1", [H, H], F32, kind="ExternalInput")
    wn1 = nc.dram_tensor("wn1", [H, H], F32, kind="ExternalInput")
    bn1 = nc.dram_tensor("bn1", [H, 1], F32, kind="ExternalInput")
    ws2 = nc.dram_tensor("ws2", [H, H], F32, kind="ExternalInput")
    wn2 = nc.dram_tensor("wn2", [H, H], F32, kind="ExternalInput")
    bn2 = nc.dram_tensor("bn2", [H, 1], F32, kind="ExternalInput")
    wl_in = nc.dram_tensor("wl_in", [H, HEADS], F32, kind="ExternalInput")
    wr_in = nc.dram_tensor("wr_in", [H, HEADS], F32, kind="ExternalInput")
    ulo_in = nc.dram_tensor("ulo_in", [128, H], BF16, kind="ExternalInput")
    uhi_in = nc.dram_tensor("uhi_in", [128, H], BF16, kind="ExternalInput")
    b1p = nc.dram_tensor("b1p", [H, 1], F32, kind="ExternalInput")
    w2_in = nc.dram_tensor("w2_in", [H, C], F32, kind="ExternalInput")
    b2c = nc.dram_tensor("b2c", [C, 1], F32, kind="ExternalInput")

    out = nc.dram_tensor("out", [OWN, C], F32, kind="ExternalOutput")

    with tile.TileContext(nc) as tc:
        with (
            tc.tile_pool(name="wpool", bufs=1) as wp,
            tc.tile_pool(name="sbuf", bufs=3) as sb,
            tc.tile_pool(name="big", bufs=1) as bigp,
            tc.tile_pool(name="psum", bufs=2, space="PSUM") as pp,
            tc.tile_pool(name="dram", bufs=1, space="DRAM") as dram,
        ):
            # ---- constants / weights resident in SBUF ----
            w_emb = wp.tile([IN, H], BF16)
            nc.sync.dma_start(w_emb[:], wemb[:])
            b_embr = wp.tile([1, H], F32)
            nc.sync.dma_start(b_embr[:], bembr[:])
            b_embc = wp.tile([H, 1], F32)
            nc.sync.dma_start(b_embc[:], bembc[:])
            w_s1 = wp.tile([H, H], F32); nc.sync.dma_start(w_s1[:], ws1[:])
            w_n1 = wp.tile([H, H], F32); nc.sync.dma_start(w_n1[:], wn1[:])
            b_n1 = wp.tile([H, 1], F32); nc.sync.dma_start(b_n1[:], bn1[:])
            w_s2 = wp.tile([H, H], F32); nc.sync.dma_start(w_s2[:], ws2[:])
            w_n2 = wp.tile([H, H], F32); nc.sync.dma_start(w_n2[:], wn2[:])
            b_n2 = wp.tile([H, 1], F32); nc.sync.dma_start(b_n2[:], bn2[:])
            w_lr = wp.tile([H, 2 * HEADS], F32)
            nc.sync.dma_start(w_lr[:, 0:HEADS], wl_in[:])
            nc.sync.dma_start(w_lr[:, HEADS:], wr_in[:])
            u_lo = wp.tile([128, H], BF16); nc.sync.dma_start(u_lo[:], ulo_in[:])
            u_hi = wp.tile([128, H], BF16); nc.sync.dma_start(u_hi[:], uhi_in[:])
            b_1p = wp.tile([H, 1], F32); nc.sync.dma_start(b_1p[:], b1p[:])
            w_2 = wp.tile([H, C], F32); nc.sync.dma_start(w_2[:], w2_in[:])
            b_2 = wp.tile([C, 1], F32); nc.sync.dma_start(b_2[:], b2c[:])

            ones1 = wp.tile([1, 128], F32)
            nc.vector.memset(ones1[:], 1.0)
            iota_sb = wp.tile([128, 128], I16)
            nc.sync.dma_start(iota_sb[:], iota_in[:])
            epsz = wp.tile([1, HEADS * H + HEADS], F32)
            nc.vector.memset(epsz[:, 0:HEADS * H], 0.0)
            nc.vector.memset(epsz[:, HEADS * H:], 1e-20)
            id64f = wp.tile([64, 64], F32)
            make_identity(nc, id64f[:])
            id128f = wp.tile([128, 128], F32)
            make_identity(nc, id128f[:])
            id128b = wp.tile([128, 128], BF16)
            nc.vector.tensor_copy(id128b[:], id128f[:])
            id40f = wp.tile([40, 40], F32)
            make_identity(nc, id40f[:])

            # deginv: per-partition scalar per block -> SBUF [128, NBLK]
            dgi_sb = bigp.tile([128, NBLK], F32)
            nc.sync.dma_start(dgi_sb[:], dgi_in[:].rearrange("b p one -> p (b one)"))

            # persistent feature planes
            h1T = bigp.tile([H, OWN], F32, tag="hT", bufs=2)  # feat-major planes
            h2T = bigp.tile([H, OWN], F32, tag="hT", bufs=2)
            h3T = bigp.tile([H, OWN], F32, tag="hT", bufs=2)
            neighT = bigp.tile([H, OWN], F32)
            er_all = bigp.tile([128, NBLK, HEADS], BF16)
            og_nm = bigp.tile([128, NBLK, 2 * H * 2], BF16)  # node-major GAT out

            # DRAM tables. SAGE tables pack 2 nodes per 256B row, so the
            # compact [*, 64] AllGather output IS the gather table. The GAT
            # table needs 68 cols/node -> 256B rows + a repack after the AG.
            tab1 = dram.tile([HALFR, D], BF16)
            mine2 = dram.tile([OWN, 64], FP8)
            ag1a = dram.tile([NID // 2, 64], FP8, addr_space="Shared")
            ag1b = dram.tile([NID // 2, 64], FP8, addr_space="Shared")
            tab2q = dram.tile([NCORES, OWN // 2, 256], FP8)
            mineg = dram.tile([OWN, 72], FP8)
            agga = dram.tile([NID // 2, 72], FP8, addr_space="Shared")
            aggb = dram.tile([NID // 2, 72], FP8, addr_space="Shared")
            tabg_e = dram.tile([NID // 2, 256], FP8)
            tabga = dram.tile([4, OWN, 256], FP8)
            tabgb = dram.tile([4, OWN, 256], FP8)
            tab1v = tab1[:]

            # ================= P1: embed =================
            # full table (replicated): tab1 row r = bf16(h1 of nodes 2r, 2r+1)
            for ch2 in range(NCH_ALL // 2):
                xb = sb.tile([IN, 2 * CH], BF16, tag="xb")
                nc.gpsimd.dma_start(xb[:], xT[:, ch2 * 2 * CH:(ch2 + 1) * 2 * CH])
                stg = sb.tile([128, 8, H], BF16, tag="stg1")
                for sub in range(2):
                    pe = pp.tile([128, 4, H], F32, space="PSUM", tag="psA", bufs=4)
                    for q in range(4):
                        nc.tensor.matmul(
                            pe[:, q, :],
                            xb[:, sub * CH + q * 128:sub * CH + (q + 1) * 128],
                            w_emb[:], start=True, stop=False)
                        nc.tensor.matmul(pe[:, q, :], ones1[0:1, 0:128],
                                         b_embr[0:1, :], start=False, stop=True)
                    nc.vector.tensor_copy(stg[:, sub * 4:(sub + 1) * 4, :], pe[:])
                nc.scalar.dma_start(
                    tab1[ch2 * CH:(ch2 + 1) * CH, :].rearrange(
                        "r (two d) -> (r two) d", two=2).rearrange(
                        "(p q) d -> p q d", q=8), stg[:])
            # own features, feat-major (f32)
            for ch in range(NCH_OWN):
                xb2 = sb.tile([IN, CH], BF16, tag="xb")
                nc.gpsimd.dma_start(xb2[:], xo[:, ch * CH:(ch + 1) * CH])
                ph = pp.tile([H, CH], F32, space="PSUM", tag="psB", bufs=4)
                nc.tensor.matmul(ph[:], w_emb[:], xb2[:], start=True, stop=True)
                nc.scalar.activation(h1T[:, ch * CH:(ch + 1) * CH], ph[:],
                                     mybir.ActivationFunctionType.Identity,
                                     bias=b_embc[:], scale=1.0)

            # ============== SAGE layer helper ==============
            def sage_agg(table, chunk_cb=None):
                """Aggregate neighbor means into neighT (feat-major, f32).

                chunk_cb(ch) runs after each 4-block group's neighT is ready so
                the dense layer + row writes overlap the remaining gathers.
                Table is parity-packed: tiles 0:TPH hold even-src slots (cols
                0:H of the gathered rows), tiles TPH:TPB odd-src (cols H:2H).
                """
                gdt = table.dtype
                delem = 256 if gdt == FP8 else D
                it4 = None
                for b in range(NBLK):
                    if b % 4 == 0:
                        it4 = sb.tile([128, 4, 2, S16], I16, tag="it", bufs=2)
                        nc.sync.dma_start(it4[:], idx_in[b:b + 4].rearrange(
                            "q p h s -> p q h s"))
                    it = it4
                    dc = sb.tile([128, TPB], I16, tag="dc", bufs=4)
                    nc.sync.dma_start(dc[:], dcode_in[b])
                    mk = sb.tile([128, TPB * 128], FP8, tag="mk", bufs=4)
                    nc.vector.tensor_tensor(
                        mk[:].rearrange("p (t d) -> p t d", t=TPB),
                        dc[:].rearrange("p (t o) -> p t o", o=1).to_broadcast(
                            [128, TPB, 128]),
                        iota_sb[:].rearrange("p (o d) -> p o d", o=1).to_broadcast(
                            [128, TPB, 128]),
                        mybir.AluOpType.is_equal)
                    g = sb.tile([128, TPB, delem], gdt, tag="g", bufs=5)
                    nc.gpsimd.dma_gather(g[:, 0:TPH, :], table,
                                         it[:, b % 4, 0, :], SLOTH, SLOTH, delem)
                    nc.gpsimd.dma_gather(g[:, TPH:TPB, :], table,
                                         it[:, b % 4, 1, :], SLOTH, SLOTH, delem)
                    pa = pp.tile([128, H], F32, space="PSUM", tag="psA", bufs=4)
                    for t in range(TPB):
                        off = 0 if t < TPH else H
                        nc.tensor.matmul(pa[:], mk[:, t * 128:(t + 1) * 128],
                                         g[:, t, off:off + H],
                                         start=(t == 0), stop=(t == TPB - 1))
                    nb = sb.tile([128, H], F32, tag="nb")
                    nc.scalar.activation(nb[:], pa[:],
                                         mybir.ActivationFunctionType.Identity,
                                         scale=dgi_sb[:, b:b + 1])
                    pt = pp.tile([H, 128], F32, space="PSUM", tag="psA", bufs=4)
                    nc.tensor.transpose(pt[:], nb[:], id128f[:])
                    nc.scalar.activation(neighT[:, b * 128:(b + 1) * 128], pt[:],
                                         mybir.ActivationFunctionType.Identity)
                    if chunk_cb is not None and b % 4 == 3:
                        chunk_cb(b // 4)

            def write_row_blk(hT_src, stg4, q, b, with_el):
                """Transpose one block's feat-major features into rows."""
                ptr = pp.tile([128, H], F32, space="PSUM", tag="psA", bufs=4)
                nc.tensor.transpose(ptr[:], hT_src[:, b * 128:(b + 1) * 128],
                                    id64f[:])
                nc.scalar.activation(stg4[:, q, 0:H], ptr[:],
                                     mybir.ActivationFunctionType.Identity)
                if with_el:
                    pel = pp.tile([128, 2 * HEADS], F32, space="PSUM", tag="psA",
                                  bufs=4)
                    nc.tensor.matmul(pel[:], hT_src[:, b * 128:(b + 1) * 128],
                                     w_lr[:], start=True, stop=True)
                    nc.vector.tensor_copy(
                        stg4[:, q, H:H + 2 * HEADS].bitcast(BF16),
                        pel[:, 0:HEADS])
                    nc.vector.tensor_copy(er_all[:, b, :], pel[:, HEADS:])

            def ag_chunk(mine, agc, j):
                """AllGather chunk j: every rank's rows [j*OWN/2,(j+1)*OWN/2)
                concatenate into the contiguous chunk buffer."""
                nc.gpsimd.collective_compute(
                    "AllGather", mybir.AluOpType.bypass,
                    replica_groups=[list(range(NCORES))],
                    ins=[mine[j * OWN // 2:(j + 1) * OWN // 2, :].opt()],
                    outs=[agc[:].opt()],
                )

            def sage_fused_cb(hT_in, w_s, w_n, b_n, hT_out, mine, width,
                              with_el=False, on_c0=None):
                def cb(ch):
                    pd = pp.tile([H, CH], F32, space="PSUM", tag="psB", bufs=4)
                    nc.tensor.matmul(pd[:], w_s[:], hT_in[:, ch * CH:(ch + 1) * CH],
                                     start=True, stop=False)
                    nc.tensor.matmul(pd[:], w_n[:], neighT[:, ch * CH:(ch + 1) * CH],
                                     start=False, stop=True)
                    nc.scalar.activation(hT_out[:, ch * CH:(ch + 1) * CH], pd[:],
                                         mybir.ActivationFunctionType.Relu,
                                         bias=b_n[:], scale=1.0)
                    stg4 = sb.tile([128, 4, width], mine.dtype, tag="stg2",
                                   bufs=2)
                    for q in range(4):
                        write_row_blk(hT_out, stg4, q, ch * 4 + q, with_el)
                    nc.sync.dma_start(
                        mine[ch * CH:(ch + 1) * CH, :].rearrange(
                            "(q p) w -> p q w", p=128), stg4[:])
                    if on_c0 is not None and ch == NCH_OWN // 2 + 1:
                        on_c0()
                return cb

            def sage_agg_el(tabE, tabL, idx2, mask2, chunk_cb):
                """Early/late variant: tiles 0:TE p0-early, TE:2TE p1-early
                (sources living in AG chunk 0, read from tabE), then TL-tile
                late groups per parity from the full table tabL."""
                for b in range(NBLK):
                    it = sb.tile([128, 128], I16, tag="it2", bufs=3)
                    nc.sync.dma_start(it[:], idx2[b])
                    mk = sb.tile([128, TPB * 128], FP8, tag="mk", bufs=4)
                    nc.sync.dma_start(mk[:], mask2[b])
                    g = sb.tile([128, TPB, D], BF16, tag="g", bufs=5)
                    nc.gpsimd.dma_gather(g[:, 0:2 * TE, :], tabE,
                                         it[:, 0:NE2 // 16], NE2, NE2, D)
                    nc.gpsimd.dma_gather(g[:, 2 * TE:2 * TE + TL, :], tabL,
                                         it[:, NE2 // 16:(NE2 + NL) // 16], NL, NL, D)
                    nc.gpsimd.dma_gather(g[:, 2 * TE + TL:TPB, :], tabL,
                                         it[:, (NE2 + NL) // 16:(NE2 + 2 * NL) // 16],
                                         NL, NL, D)
                    pa = pp.tile([128, H], F32, space="PSUM", tag="psA", bufs=4)
                    for t in range(TPB):
                        off = 0 if (t < TE or 2 * TE <= t < 2 * TE + TL) else H
                        nc.tensor.matmul(pa[:], mk[:, t * 128:(t + 1) * 128],
                                         g[:, t, off:off + H],
                                         start=(t == 0), stop=(t == TPB - 1))
                    nb = sb.tile([128, H], F32, tag="nb")
                    nc.scalar.activation(nb[:], pa[:],
                                         mybir.ActivationFunctionType.Identity,
                                         scale=dgi_sb[:, b:b + 1])
                    pt = pp.tile([H, 128], F32, space="PSUM", tag="psA", bufs=4)
                    nc.tensor.transpose(pt[:], nb[:], id128f[:])
                    nc.scalar.activation(neighT[:, b * 128:(b + 1) * 128], pt[:],
                                         mybir.ActivationFunctionType.Identity)
                    if chunk_cb is not None and b % 4 == 3:
                        chunk_cb(b // 4)

            # ================= SAGE 1 =================
            def sage1_c0():
                ag_chunk(mine2, ag1a, 0)
                nc.sync.dma_start(
                    tab2q[:, 0:OWN // 4, 0:128],
                    ag1a[:].rearrange("(c r two) d -> c r (two d)",
                                      c=NCORES, two=2))
            if upto >= 2:
                sage_agg(tab1v,
                         sage_fused_cb(h1T, w_s1, w_n1, b_n1, h2T, mine2, 64,
                                       on_c0=sage1_c0)
                         if upto >= 3 else None)
            if upto >= 3:
                ag_chunk(mine2, ag1b, 1)
                nc.sync.dma_start(
                    tab2q[:, OWN // 4:OWN // 2, 0:128],
                    ag1b[:].rearrange("(c r two) d -> c r (two d)",
                                      c=NCORES, two=2))

            # ================= SAGE 2 =================
            def sage2_c0():
                ag_chunk(mineg, agga, 0)
                # early GAT table: all chunk-0 rows (fits int16 indexing)
                nc.scalar.dma_start(tabg_e[:, 0:72], agga[:])
            if upto >= 4:
                sage_agg(tab2q[:].rearrange("c r d -> (c r) d"),
                         sage_fused_cb(h2T, w_s2, w_n2, b_n2, h3T, mineg, 72,
                                       with_el=True, on_c0=sage2_c0))
                ag_chunk(mineg, aggb, 1)
                # unified GAT tables from both chunks
                nc.sync.dma_start(
                    tabga[:, 0:OWN // 2, 0:72],
                    agga[0:HALFR // 2, :].rearrange("(c r) d -> c r d", c=4))
                nc.scalar.dma_start(
                    tabgb[:, 0:OWN // 2, 0:72],
                    agga[HALFR // 2:NID // 2, :].rearrange("(c r) d -> c r d", c=4))
                nc.sync.dma_start(
                    tabga[:, OWN // 2:OWN, 0:72],
                    aggb[0:HALFR // 2, :].rearrange("(c r) d -> c r d", c=4))
                nc.scalar.dma_start(
                    tabgb[:, OWN // 2:OWN, 0:72],
                    aggb[HALFR // 2:NID // 2, :].rearrange("(c r) d -> c r d", c=4))

            # ------- GAT dense + classifier (per 4-block chunk) -------
            def og_stage(ch, half):
                stgT = sb.tile([128, CH], BF16, tag=f"ogs{half}", bufs=2)
                for q in range(4):
                    b = ch * 4 + q
                    ptg = pp.tile([128, 128], BF16, space="PSUM", tag="psA", bufs=4)
                    nc.tensor.transpose(
                        ptg[:], og_nm[:, b, half * 128:(half + 1) * 128], id128b[:])
                    nc.vector.tensor_copy(stgT[:, q * 128:(q + 1) * 128], ptg[:])
                return stgT

            def gat_tail(ch):
                og_loS = og_stage(ch, 0)
                og_hiS = og_stage(ch, 1)
                p4 = pp.tile([H, CH], F32, space="PSUM", tag="psB", bufs=4)
                nc.tensor.matmul(p4[:], u_lo[:], og_loS[:],
                                 start=True, stop=False)
                nc.tensor.matmul(p4[:], u_hi[:], og_hiS[:],
                                 start=False, stop=True)
                h4 = sb.tile([H, CH], F32, tag="h4")
                nc.scalar.activation(h4[:], p4[:],
                                     mybir.ActivationFunctionType.Relu,
                                     bias=b_1p[:], scale=1.0)
                plg = pp.tile([C, CH], F32, space="PSUM", tag="psB", bufs=4)
                nc.tensor.matmul(plg[:], w_2[:], h4[:], start=True, stop=True)
                lg = sb.tile([C, CH], F32, tag="lg")
                nc.scalar.activation(lg[:], plg[:],
                                     mybir.ActivationFunctionType.Identity,
                                     bias=b_2[:], scale=1.0)
                ostg = sb.tile([128, 4, C], F32, tag="ostg")
                for q in range(4):
                    plt = pp.tile([128, C], F32, space="PSUM", tag="psA", bufs=4)
                    nc.tensor.transpose(plt[:], lg[:, q * 128:(q + 1) * 128], id40f[:])
                    nc.scalar.activation(ostg[:, q, :], plt[:],
                                         mybir.ActivationFunctionType.Identity)
                nc.sync.dma_start(
                    out[ch * CH:(ch + 1) * CH, :].rearrange("(q p) c -> p q c", p=128),
                    ostg[:])

            # ================= GAT aggregation =================
            _noW = _noER = _noPG = _noTR = False
            tabgaf = tabga[:].rearrange("c r d -> (c r) d")
            tabgbf = tabgb[:].rearrange("c r d -> (c r) d")
            for b in range(NBLK if upto >= 5 else 0):
                it4g = sb.tile([128, 128], I16, tag="it2", bufs=3)
                nc.sync.dma_start(it4g[:], idx_g_in[b])
                mk = sb.tile([128, TPB * 128], FP8, tag="mk", bufs=4)
                nc.sync.dma_start(mk[:], mask_g_in[b])
                mt = sb.tile([128, TPB * 128], FP8, tag="mt", bufs=3)
                nc.sync.dma_start(mt[:], maskT_in[b])
                g = sb.tile([128, TPB, 256], FP8, tag="g", bufs=5)
                nc.gpsimd.dma_gather(g[:, 0:2 * TE, :], tabg_e[:],
                                     it4g[:, 0:NE2 // 16], NE2, NE2, 256)
                nc.gpsimd.dma_gather(g[:, 2 * TE:2 * TE + TL, :], tabgaf,
                                     it4g[:, NE2 // 16:(NE2 + NL) // 16],
                                     NL, NL, 256)
                nc.gpsimd.dma_gather(g[:, 2 * TE + TL:TPB, :], tabgbf,
                                     it4g[:, (NE2 + NL) // 16:128],
                                     NL, NL, 256)
                # er broadcast to edge slots via maskT matmuls
                perb = pp.tile([128, TPB, HEADS], F32, space="PSUM", tag="psA", bufs=4)
                for t in range(TPB):
                    nc.tensor.matmul(perb[:, t, :], mt[:, t * 128:(t + 1) * 128],
                                     er_all[:, b, :], start=True, stop=True)
                # e = leaky_relu(el + er); ex = exp(e)  (no Lrelu table: max(x, .2x))
                ee = sb.tile([128, TPB, HEADS], BF16, tag="ee")
                nc.vector.tensor_add(ee[:], g[:, :, H:H + 2 * HEADS].bitcast(BF16), perb[:])
                eeL = sb.tile([128, TPB, HEADS], BF16, tag="et")
                nc.scalar.activation(eeL[:], ee[:],
                                     mybir.ActivationFunctionType.Prelu,
                                     alpha=SLOPE)
                wst = sb.tile([128, TPB, HEADS * H + HEADS], BF16, tag="wst", bufs=3)
                nc.scalar.activation(wst[:, :, HEADS * H:], eeL[:],
                                     mybir.ActivationFunctionType.Exp)
                pg = pp.tile([128, HEADS * H + HEADS], F32, space="PSUM", tag="psB", bufs=4)
                HT = TPB // 2
                for half in range(2):
                    tsl = slice(half * HT, (half + 1) * HT)
                    if _noW:
                        if b == 0 and half == 0:
                            nc.vector.memset(wst[:, :, 0:HEADS * H], 0.5)
                    else:
                        # fused per-head weighting: wst[p,t,h,f] = g[p,t,f]*ex[p,t,h]
                        nc.vector.tensor_mul(
                            wst[:, tsl, 0:HEADS * H].rearrange(
                                "p t (h f) -> p t h f", h=HEADS),
                            g[:, tsl, 0:H].rearrange(
                                "p t (o f) -> p t o f", o=1).to_broadcast(
                                    [128, HT, HEADS, H]),
                            wst[:, tsl, HEADS * H:].rearrange(
                                "p t (h o) -> p t h o", o=1).to_broadcast(
                                    [128, HT, HEADS, H]))
                    for t in range(half * HT, (half + 1) * HT):
                        nc.tensor.matmul(pg[:], mk[:, t * 128:(t + 1) * 128],
                                         wst[:, t, :], start=(t == 0),
                                         stop=False)
                nc.tensor.matmul(pg[:], ones1[0:1, 0:128],
                                 epsz[0:1, :], start=False, stop=True)
                # normalize by z and transpose for the dense phase
                zi = sb.tile([128, HEADS], F32, tag="zi")
                nc.vector.reciprocal(zi[:], pg[:, HEADS * H:])
                nc.vector.tensor_mul(
                    og_nm[:, b, :].rearrange("p (h f) -> p h f", h=HEADS),
                    pg[:, 0:HEADS * H].rearrange("p (h f) -> p h f", h=HEADS),
                    zi[:].to_broadcast([128, HEADS, H]))
            for ch in range(NCH_OWN if upto >= 6 else 0):
                gat_tail(ch)

            if upto < 6:
                zo = sb.tile([128, NBLK, C], F32, tag="zo")
                nc.vector.memset(zo[:], 0.0)
                nc.sync.dma_start(
                    out[:].rearrange("(q p) c -> p q c", p=128), zo[:])

    nc.compile()
    return nc


def _plan(src, dst):
    """Host-side graph partitioning. Returns per-core index/mask arrays."""
    src = np.asarray(src).astype(np.int64)
    dst = np.asarray(dst).astype(np.int64)

    def grouping(si, di, half, val):
        """Slot layout for one (half-assignment, idx-value) scheme."""
        gblk = di // PB
        grp = gblk * 2 + half
        cnt = np.bincount(grp, minlength=NCORES * NBLK * 2)
        if cnt.max() > SLOTH:
            return None
        order = np.lexsort((si, grp))
        g_sorted = grp[order]
        starts = np.zeros(NCORES * NBLK * 2 + 1, np.int64)
        np.cumsum(cnt, out=starts[1:])
        j_in_grp = np.arange(E, dtype=np.int64) - starts[g_sorted]
        e_di = di[order]
        e_half = half[order]
        e_gblk = gblk[order]
        e_core = e_gblk // NBLK
        e_blk = e_gblk % NBLK

        idx16 = np.zeros((NCORES, NBLK, 16, 2, S16), np.int16)
        idx16[e_core, e_blk, j_in_grp % 16, e_half, j_in_grp // 16] = \
            val[order].astype(np.int16)
        idx16 = np.broadcast_to(idx16[:, :, None, :, :, :],
                                (NCORES, NBLK, 8, 16, 2, S16)).reshape(
                                    NCORES, NBLK, 128, 2, S16).copy()

        t_of = (e_half * TPH + j_in_grp // 128).astype(np.int64)
        p_of = (j_in_grp % 128).astype(np.int64)
        d_of = (e_di % PB).astype(np.int64)
        # dst codes: one dst column (or 255 = empty slot) per (partition, tile)
        dcode = np.full((NCORES, NBLK, 128, TPB), 255, np.int16)
        dcode[e_core, e_blk, p_of, t_of] = d_of.astype(np.int16)
        return idx16, dcode

    def grouping_el(si, di, half, jcls, val_e, val_l, early_merged, with_maskT):
        """Early/late slot layout. Early tiles hold only chunk-0 (jcls==0)
        sources; overflow and all chunk-1 sources go to the late tiles."""
        CAP_E, CAP_L = TE * 128, TL * 128
        NG = NCORES * NBLK * 2
        gblk = di // PB
        key = gblk * 2 + half
        order = np.lexsort((si, jcls, key))
        k_s = key[order]
        cnt = np.bincount(key, minlength=NG)
        starts = np.zeros(NG + 1, np.int64)
        np.cumsum(cnt, out=starts[1:])
        r = np.arange(E, dtype=np.int64) - starts[k_s]
        j_s = jcls[order]
        j0cnt = np.bincount(key[jcls == 0], minlength=NG)
        etk = np.minimum(j0cnt, CAP_E)
        is_e = (j_s == 0) & (r < CAP_E)
        lr = r - etk[k_s]
        if int(np.where(is_e, 0, lr).max()) >= CAP_L:
            return None
        e_half = half[order]
        e_di = di[order]
        e_gblk = gblk[order]
        e_core = e_gblk // NBLK
        e_blk = e_gblk % NBLK

        tile = np.where(is_e, e_half * TE + r // 128,
                        2 * TE + e_half * TL + lr // 128)
        pos = np.where(is_e, r % 128, lr % 128)
        m8 = np.zeros((NCORES, NBLK, 128, TPB * 128), np.uint8)
        one_fp8 = np.array(1.0, NP_FP8).view(np.uint8).item()
        m8[e_core, e_blk, pos, tile * 128 + (e_di % PB)] = one_fp8
        mT8 = None
        if with_maskT:
            mT8 = m8.reshape(NCORES, NBLK, 128, TPB, 128).transpose(0, 1, 4, 3, 2)
            mT8 = np.ascontiguousarray(mT8).reshape(
                NCORES, NBLK, 128, TPB * 128).view(NP_FP8)

        # idx streams -> [.., 16ch, 128 cols] wrapped layout
        if early_merged:
            ej = e_half * CAP_E + r          # one gather, cols [0, 2*CAP_E/16)
            ecol, lcol0 = ej // 16, 2 * CAP_E // 16
            ech = ej % 16
        else:
            ecol = e_half * (CAP_E // 16) + r // 16
            ech = r % 16
            lcol0 = 2 * (CAP_E // 16)
        lj = lr
        col = np.where(is_e, ecol, lcol0 + e_half * (CAP_L // 16) + lj // 16)
        chan = np.where(is_e, ech, lj % 16)
        v = np.where(is_e, val_e[order], val_l[order]).astype(np.int16)
        idx16 = np.zeros((NCORES, NBLK, 16, 128), np.int16)
        idx16[e_core, e_blk, chan, col] = v
        idx16 = np.broadcast_to(idx16[:, :, None, :, :],
                                (NCORES, NBLK, 8, 16, 128)).reshape(
                                    NCORES, NBLK, 128, 128).copy()
        return idx16, m8.view(NP_FP8), mT8

    for seed in range(64):
        rng = np.random.default_rng(seed)
        perm = rng.permutation(NID)[:N].astype(np.int64)  # orig -> internal
        si = perm[src]
        di = perm[dst]
        gs = grouping(si, di, si % 2, si // 2)                  # SAGE: parity
        cc, kk = si // OWN, si % OWN
        jcls = (kk >= OWN // 2).astype(np.int64)
        half_g = (si >= HALFR).astype(np.int64)
        vEg = cc * (OWN // 2) + (kk % (OWN // 2))   # row in unified tabg_e
        vLg = si - half_g * HALFR
        gg = grouping_el(si, di, half_g, jcls, vEg, vLg, True, True)
        if gs is not None and gg is not None:
            break
    else:
        raise RuntimeError("could not pack edges into halves; increase NBLK")

    # deginv per dst slot
    deg = np.bincount(di, minlength=NID).astype(np.float32)
    dgi = (1.0 / np.maximum(deg, 1.0)).reshape(NCORES, NBLK, PB, 1)

    return perm, gs[0], gs[1], gg[0], gg[1], gg[2], dgi


def kernel(x, src, dst, W_embed, b_embed, Ws1, Wn1, bn1, Ws2, Wn2, bn2,
           Wg, al, ar, bg, W1, b1, W2, b2):
    x = np.asarray(x, np.float32)
    perm, idxS, dcS, idxG, mG, mTG, dgi = _plan(src, dst)

    if "nc" not in _cached:
        _cached["nc"] = _build_bass()
    nc = _cached["nc"]

    # weight preprocessing
    Wg = np.asarray(Wg, np.float32)
    al = np.asarray(al, np.float32)
    ar = np.asarray(ar, np.float32)
    W1 = np.asarray(W1, np.float32)
    WL = np.stack([Wg[:, h * H:(h + 1) * H] @ al[h] for h in range(HEADS)], 1)
    WR = np.stack([Wg[:, h * H:(h + 1) * H] @ ar[h] for h in range(HEADS)], 1)
    b1p = (np.asarray(b1, np.float32) + np.asarray(bg, np.float32) @ W1)
    U = [Wg[:, h * H:(h + 1) * H] @ W1[h * H:(h + 1) * H] for h in range(HEADS)]
    Ulo = np.vstack([U[0], U[1]]).astype(NP_BF16)
    Uhi = np.vstack([U[2], U[3]]).astype(NP_BF16)

    xT = np.zeros((IN, NID), np.float32)
    xT[:, perm] = x.T
    # embed writes table rows p-major (row = p*8+q within each 1024-chunk) so
    # the DMA emits 1KB descriptors; present xT columns in matching order
    xTs = np.ascontiguousarray(
        np.swapaxes(xT.reshape(IN, NCH_ALL // 2, 128, 8), 2, 3).reshape(IN, NID))

    common = {
        "xT": xTs.astype(NP_BF16),
        "wemb": np.asarray(W_embed, np.float32).astype(NP_BF16),
        "bembr": np.asarray(b_embed, np.float32).reshape(1, H),
        "bembc": np.asarray(b_embed, np.float32).reshape(H, 1),
        "ws1": np.asarray(Ws1, np.float32), "wn1": np.asarray(Wn1, np.float32),
        "bn1": np.asarray(bn1, np.float32).reshape(H, 1),
        "ws2": np.asarray(Ws2, np.float32), "wn2": np.asarray(Wn2, np.float32),
        "bn2": np.asarray(bn2, np.float32).reshape(H, 1),
        "wl_in": WL, "wr_in": WR,
        "ulo_in": Ulo, "uhi_in": Uhi,
        "b1p": b1p.reshape(H, 1),
        "w2_in": np.asarray(W2, np.float32),
        "b2c": np.asarray(b2, np.float32).reshape(C, 1),
        "iota_in": np.broadcast_to(np.arange(128, dtype=np.int16), (128, 128)).copy(),
    }
    in_maps = []
    for c in range(NCORES):
        m = dict(common)
        m["xo"] = np.ascontiguousarray(xT[:, c * OWN:(c + 1) * OWN]).astype(NP_BF16)
        m["idx_in"] = np.ascontiguousarray(idxS[c])
        m["dcode_in"] = np.ascontiguousarray(dcS[c])
        m["idx_g_in"] = np.ascontiguousarray(idxG[c])
        m["mask_g_in"] = np.ascontiguousarray(mG[c])
        m["maskT_in"] = np.ascontiguousarray(mTG[c])
        m["dgi_in"] = np.ascontiguousarray(dgi[c])
        in_maps.append(m)

    res = run_bass_kernel_spmd(nc, in_maps, core_ids=list(range(NCORES)))
    full = np.concatenate([res.results[c]["out"] for c in range(NCORES)], 0)
    return full[perm].astype(np.float32)

